# revision 21
# baseline (speedup 1.0000x reference)
"""Trainium2 Bass kernel for nn_MultiHeadAttention (B=2, L=2048, H=768, 12 heads).

Sharding (8 cores): core c -> batch b=c//4, heads 3*(c%4)..3*(c%4)+2.
Each core: QKV proj for its 3 heads, flash-style attention (scores^T layout,
key-mask folded into V', query-mask folded into 1/l), partial output
projection with wo rows (row-parallel) + x/4 residual, ReduceScatter(add)
over the 4 cores of its batch, then layernorm over the sequence dim on its
192-row hidden slice. Host assembles [2,2048,768] from 8 [192,2048] slices.

PSUM static budget (8 banks): tag s = 2 bufs x [128,1024] (4 banks, shared by
scores / transposes / projections), tag av = [65,1024] (2), tag rb = [64,1024]
(2).
"""

import sys

import numpy as np

sys.path.insert(0, "/opt/trn_rl_repo")

import concourse.bass as bass  # noqa: E402
import concourse.bacc as bacc  # noqa: E402
import concourse.mybir as mybir  # noqa: E402
from concourse import tile  # noqa: E402
from concourse.bass_utils import run_bass_kernel_spmd  # noqa: E402

F32 = mybir.dt.float32
BF16 = mybir.dt.bfloat16
I32 = mybir.dt.int32
AF = mybir.ActivationFunctionType
ALU = mybir.AluOpType

HIDDEN = 768
HEADS = 12
HD = 64
L = 2048
B = 2
NCORES = 8
HPC = 3          # heads per core
HF = HPC * HD    # 192 features per core
LT = L // 128    # 16 l-tiles
HC = HIDDEN // 128  # 6 hidden chunks
OSL = HIDDEN // 4   # 192 output-slice rows per core


def build_nc():
    nc = bacc.Bacc("TRN2", target_bir_lowering=False, debug=False,
                   num_devices=NCORES)

    x_d = nc.dram_tensor("x", [L, HIDDEN], F32, kind="ExternalInput")
    wq_d = nc.dram_tensor("wq", [HIDDEN, HF], F32, kind="ExternalInput")
    wk_d = nc.dram_tensor("wk", [HIDDEN, HF], F32, kind="ExternalInput")
    wv_d = nc.dram_tensor("wv", [HIDDEN, HF], F32, kind="ExternalInput")
    wo_d = nc.dram_tensor("wo_r", [HF, HIDDEN], F32, kind="ExternalInput")
    mask_d = nc.dram_tensor("mask_i", [1, L], I32, kind="ExternalInput")
    # params_col[128, 16]: cols 0,1=wq_b(192) 2,3=wk_b 4,5=wv_b 6..11=wo_b/4
    # (768), 12,13=gamma slice, 14,15=beta slice
    pcol_d = nc.dram_tensor("params_col", [128, 16], F32, kind="ExternalInput")
    # params_row[1, 960]: 0:192 wv_b, 192:960 wo_b/4
    prow_d = nc.dram_tensor("params_row", [1, 960], F32, kind="ExternalInput")
    out_d = nc.dram_tensor("out_t", [OSL, L], F32, kind="ExternalOutput")

    partial_d = nc.dram_tensor("partial_acc", [HIDDEN, L], F32)
    rs_d = nc.dram_tensor("rs_out", [OSL * L], F32)

    with tile.TileContext(nc) as tc:
        with (
            tc.tile_pool(name="persist", bufs=1) as pers,
            tc.tile_pool(name="xin", bufs=3) as xin,
            tc.tile_pool(name="work", bufs=2) as work,
            tc.tile_pool(name="ps2", bufs=2, space=bass.MemorySpace.PSUM) as ps2,
            tc.tile_pool(name="pav", bufs=1, space=bass.MemorySpace.PSUM) as pav,
            tc.tile_pool(name="prb", bufs=1, space=bass.MemorySpace.PSUM) as prb,
            tc.tile_pool(name="pexp", bufs=3) as pexp,
        ):
            def ps_tile(shape, name):
                return ps2.tile(shape, F32, tag="s", name=name,
                                padded_shape=[128, 1024])

            # ---------- phase 0: constants ----------
            ident_i = pers.tile([128, 128], I32, tag="ident_i")
            nc.gpsimd.iota(ident_i[:], pattern=[[-1, 128]], base=0,
                           channel_multiplier=1)
            ident = pers.tile([128, 128], F32, tag="ident")
            nc.vector.tensor_scalar(
                ident[:], ident_i[:], 0, None, op0=ALU.is_equal
            )
            ones_row = pers.tile([1, 512], F32, tag="ones_row")
            nc.vector.memset(ones_row[:], 1.0)

            pcol = pers.tile([128, 16], F32, tag="pcol")
            nc.sync.dma_start(out=pcol[:], in_=pcol_d[:])
            prow = pers.tile([1, 960], F32, tag="prow")
            nc.sync.dma_start(out=prow[:], in_=prow_d[:])

            mask_i = xin.tile([1, L], I32, tag="mask_i", bufs=1)
            nc.sync.dma_start(out=mask_i[:], in_=mask_d[:])
            mask_row = pers.tile([1, L], F32, tag="mask_row")
            nc.vector.tensor_copy(mask_row[:], mask_i[:])

            # mask columns [128, 16]: col t = mask[128t:128t+128]
            mask_cols = pers.tile([128, LT], F32, tag="mask_cols")
            for t in range(LT):
                mp = ps_tile([128, 1], f"mask_ps{t}")
                nc.tensor.matmul(
                    mp[:], mask_row[:, 128 * t:128 * (t + 1)], ones_row[:, 0:1]
                )
                nc.vector.tensor_copy(mask_cols[:, t:t + 1], mp[:])

            # weights loaded early; tiny PE "touch" matmuls absorb each DMA
            # lane wait so later matmuls stay under the 2-wait limit
            wq = pers.tile([128, HC, HF], F32, tag="wq")
            wk = pers.tile([128, HC, HF], F32, tag="wk")
            wv = pers.tile([128, HC, HF], F32, tag="wv")
            for w_sb, w_d in ((wq, wq_d), (wk, wk_d), (wv, wv_d)):
                nc.sync.dma_start(
                    out=w_sb[:], in_=w_d[:].rearrange("(c p) m -> p c m", p=128)
                )
            wo_a = pers.tile([128, HIDDEN], F32, tag="wo_a")
            wo_b_sb = pers.tile([64, HIDDEN], F32, tag="wo_b")
            nc.sync.dma_start(out=wo_a[:], in_=wo_d[0:128, :])
            nc.sync.dma_start(out=wo_b_sb[:], in_=wo_d[128:192, :])
            touch_srcs = (wq[:, 0, 0:1], wk[:, 0, 0:1], wv[:, 0, 0:1],
                          wo_a[:, 0:1], wo_b_sb[:, 0:1], prow[:, 0:1])
            tch = pav.tile([1, 1], F32, tag="av", name="touch",
                           padded_shape=[65, 1024])
            for ti, tsr in enumerate(touch_srcs):
                nc.tensor.matmul(tch[:], tsr, tsr, start=(ti == 0),
                                 stop=(ti == len(touch_srcs) - 1),
                                 skip_group_check=True)
            tch_scr = work.tile([1, 1], F32, tag="tch_scr", bufs=1)
            nc.scalar.copy(tch_scr[:], tch[:])

            # ---------- phase 1: load x, build x^T ----------
            x_t = [pers.tile([128, L], F32, tag=f"x_t{c}", name=f"x_t{c}")
                   for c in range(HC)]
            for lt in range(LT):
                xn = xin.tile([128, HIDDEN], F32, tag="x_nat")
                nc.gpsimd.dma_start(out=xn[:], in_=x_d[128 * lt:128 * (lt + 1), :])
                for c in range(HC):
                    tp = ps_tile([128, 128], f"tr_ps{lt}_{c}")
                    nc.tensor.transpose(tp[:], xn[:, 128 * c:128 * (c + 1)], ident[:])
                    nc.vector.tensor_copy(
                        x_t[c][:, 128 * lt:128 * (lt + 1)], tp[:]
                    )

            # ---------- phase 2: QKV projections ----------
            # q^T / k^T: [192, L] as a [128, L] + [64, L] pair
            q_a = pers.tile([128, L], F32, tag="q_a")
            k_a = pers.tile([128, L], F32, tag="k_a")
            q_b_t = pers.tile([64, L], F32, tag="q_b")
            k_b_t = pers.tile([64, L], F32, tag="k_b")
            q_b = q_b_t[:]
            k_b = k_b_t[:]
            for wi, (dst, w_sb, bcol) in enumerate((
                ((q_a[:], q_b), wq, 0),
                ((k_a[:], k_b), wk, 2),
            )):
                for fc in range(2):  # feature chunk: 0 -> 128 rows, 1 -> 64 rows
                    m = 128 if fc == 0 else 64
                    for half in range(2):
                        ps = ps_tile([m, 1024], f"qk_ps{wi}_{fc}_{half}")
                        for qt in range(2):
                            sl = slice(512 * qt, 512 * (qt + 1))
                            xsl = slice(1024 * half + 512 * qt,
                                        1024 * half + 512 * (qt + 1))
                            for c in range(HC):
                                nc.tensor.matmul(
                                    ps[:, sl],
                                    w_sb[:, c, 128 * fc:128 * fc + m],
                                    x_t[c][:, xsl],
                                    start=(c == 0),
                                    stop=(c == HC - 1),
                                )
                        nc.vector.tensor_scalar_add(
                            dst[fc][:, 1024 * half:1024 * (half + 1)], ps[:],
                            pcol[0:m, bcol + fc:bcol + fc + 1]
                        )

            # V' tiles: [128, 3*65] per l-tile; per head h cols 65h..65h+63 =
            # (x@wv + b)*mask, col 65h+64 = mask
            v_sb = [work.tile([128, 3 * 65], BF16, tag=f"v{lt}", name=f"v{lt}",
                              bufs=1)
                    for lt in range(LT)]
            for lt in range(LT):
                vp = ps_tile([128, HF], f"v_ps{lt}")
                for c in range(HC):
                    nc.tensor.matmul(
                        vp[:],
                        x_t[c][:, 128 * lt:128 * (lt + 1)],
                        wv[:, c, :],
                        start=(c == 0),
                        stop=False,
                    )
                # + wv_b broadcast over rows: ones_col^T (K=1) x bias row
                nc.tensor.matmul(
                    vp[:],
                    ones_row[:, 0:128],
                    prow[:, 0:HF],
                    start=False,
                    stop=True,
                )
                for h in range(HPC):
                    nc.vector.tensor_scalar_mul(
                        v_sb[lt][:, 65 * h:65 * h + 64],
                        vp[:, 64 * h:64 * (h + 1)],
                        mask_cols[:, lt:lt + 1],
                    )
                    nc.vector.tensor_copy(
                        v_sb[lt][:, 65 * h + 64:65 * h + 65],
                        mask_cols[:, lt:lt + 1],
                    )

            # ---------- phase 3: attention ----------
            attn_a = pers.tile([128, L], F32, tag="attn_a")  # heads 0,1
            attn_b = pers.tile([64, L], F32, tag="attn_b")   # head 2
            sp_tiles = []
            for h in range(HPC):
                if h == 0:
                    q_ap, k_ap, o_ap = q_a[0:64, :], k_a[0:64, :], attn_a[0:64, :]
                elif h == 1:
                    q_ap, k_ap, o_ap = (q_a[64:128, :], k_a[64:128, :],
                                        attn_a[64:128, :])
                else:
                    q_ap, k_ap, o_ap = q_b, k_b, attn_b[:]
                for qh in range(2):  # 1024-wide query halves
                    q0 = 1024 * qh
                    av = pav.tile([65, 1024], F32, tag="av", name=f"av{h}_{qh}")
                    for kt in range(LT):
                        sp = ps_tile([128, 1024], f"s{h}_{qh}_{kt}")
                        sp_tiles.append(sp)
                        for qq in range(2):
                            nc.tensor.matmul(
                                sp[:, 512 * qq:512 * (qq + 1)],
                                k_ap[:, 128 * kt:128 * (kt + 1)],
                                q_ap[:, q0 + 512 * qq:q0 + 512 * (qq + 1)],
                            )
                        pexp_t = pexp.tile([128, 1024], BF16, tag="p",
                                           name=f"p{h}_{qh}_{kt}")
                        nc.scalar.activation(pexp_t[:], sp[:], AF.Exp, scale=0.125)
                        for qq in range(2):
                            nc.tensor.matmul(
                                av[:, 512 * qq:512 * (qq + 1)],
                                v_sb[kt][:, 65 * h:65 * (h + 1)],
                                pexp_t[:, 512 * qq:512 * (qq + 1)],
                                start=(kt == 0),
                                stop=(kt == LT - 1),
                            )
                    # normalize: (1/l) * query-mask, broadcast over 64 partitions
                    # av drains via ACT so its slot-WAR merges with the exp
                    # wait on the next accumulation group's first matmul
                    av_sb = work.tile([65, 1024], F32, tag="av_sb", bufs=1,
                                      name=f"avs{h}_{qh}")
                    nc.scalar.copy(av_sb[:], av[:])
                    r_sb = work.tile([1, 1024], F32, tag="r_sb", bufs=1,
                                     name=f"r{h}_{qh}")
                    nc.vector.reciprocal(r_sb[:], av_sb[64:65, :])
                    nc.vector.tensor_mul(
                        r_sb[:], r_sb[:], mask_row[:, q0:q0 + 1024]
                    )
                    rb = prb.tile([64, 1024], F32, tag="rb", name=f"rb{h}_{qh}")
                    for i in range(2):
                        nc.tensor.matmul(
                            rb[:, 512 * i:512 * (i + 1)],
                            ones_row[:, 0:64],
                            r_sb[:, 512 * i:512 * (i + 1)],
                        )
                    rb_sb = work.tile([64, 1024], F32, tag="rb_sb", bufs=1,
                                      name=f"rbs{h}_{qh}")
                    nc.vector.tensor_copy(rb_sb[:], rb[:])
                    nc.vector.tensor_mul(
                        o_ap[:, q0:q0 + 1024], av_sb[0:64, :], rb_sb[:]
                    )

            # ---------- phase 4: output projection (row-parallel partial) ----
            sp_scr = work.tile([1, 1], F32, tag="sp_scr", bufs=2)
            for spt in sp_tiles[-2:]:
                scr = work.tile([1, 1], F32, tag="sp_scr", bufs=2,
                                name="sp_touch")
                nc.vector.tensor_copy(scr[:], spt[0:1, 0:1])
            for oc in range(HC):
                st = work.tile([128, L], F32, tag="stage", bufs=2,
                               name=f"st{oc}")
                for half in range(2):
                    po = ps_tile([128, 1024], f"po{oc}_{half}")
                    for qt in range(2):
                        sl = slice(512 * qt, 512 * (qt + 1))
                        asl = slice(1024 * half + 512 * qt,
                                    1024 * half + 512 * (qt + 1))
                        nc.tensor.matmul(
                            po[:, sl],
                            wo_a[:, 128 * oc:128 * (oc + 1)],
                            attn_a[:, asl],
                            start=True,
                            stop=False,
                        )
                        nc.tensor.matmul(
                            po[:, sl],
                            wo_b_sb[:, 128 * oc:128 * (oc + 1)],
                            attn_b[:, asl],
                            start=False,
                            stop=False,
                        )
                        # + wo_b/4 broadcast over columns
                        nc.tensor.matmul(
                            po[:, sl],
                            prow[:, HF + 128 * oc:HF + 128 * (oc + 1)],
                            ones_row[:, 0:512],
                            start=False,
                            stop=True,
                        )
                    # stage = x/4 + partial_out
                    nc.vector.scalar_tensor_tensor(
                        st[:, 1024 * half:1024 * (half + 1)],
                        x_t[oc][:, 1024 * half:1024 * (half + 1)],
                        0.25, po[:], op0=ALU.mult, op1=ALU.add,
                    )
                nc.gpsimd.dma_start(
                    out=partial_d[128 * oc:128 * (oc + 1), :], in_=st[:]
                )

            # ---------- phase 5: reduce-scatter + layernorm over L ----------
            nc.gpsimd.collective_compute(
                "ReduceScatter",
                ALU.add,
                replica_groups=[[0, 1, 2, 3], [4, 5, 6, 7]],
                ins=[partial_d[:].opt()],
                outs=[rs_d[:].opt()],
            )
            rs_ap = rs_d[:].rearrange("(r l) -> r l", l=L)
            for pc, m in ((0, 128), (1, 64)):
                y = work.tile([m, L], F32, tag="y", bufs=1, name=f"y{pc}")
                nc.sync.dma_start(out=y[:], in_=rs_ap[128 * pc:128 * pc + m, :])
                bnst = work.tile([m, 4 * 6], F32, tag=f"bnst{pc}", bufs=1,
                                 name=f"bnst{pc}")
                for cch in range(4):
                    nc.vector.bn_stats(
                        bnst[:, 6 * cch:6 * (cch + 1)],
                        y[:, 512 * cch:512 * (cch + 1)],
                    )
                stats = work.tile([m, 2], F32, tag=f"stats{pc}", bufs=1,
                                  name=f"stats{pc}")
                nc.vector.bn_aggr(stats[:], bnst[:])
                std = work.tile([m, 1], F32, tag=f"std{pc}", bufs=1,
                                name=f"std{pc}")
                nc.scalar.activation(
                    std[:], stats[:, 1:2], AF.Sqrt, scale=float(L) / float(L - 1)
                )
                rstd = work.tile([m, 1], F32, tag=f"rstd{pc}", bufs=1,
                                 name=f"rstd{pc}")
                nc.vector.reciprocal(rstd[:], std[:])
                ga = pcol[0:m, 12 + pc:13 + pc]
                be = pcol[0:m, 14 + pc:15 + pc]
                amul = work.tile([m, 1], F32, tag=f"amul{pc}", bufs=1,
                                 name=f"amul{pc}")
                nc.vector.tensor_mul(amul[:], rstd[:], ga)
                tmpb = work.tile([m, 1], F32, tag=f"tmpb{pc}", bufs=1,
                                 name=f"tmpb{pc}")
                nc.vector.tensor_mul(tmpb[:], stats[:, 0:1], amul[:])
                badd = work.tile([m, 1], F32, tag=f"badd{pc}", bufs=1,
                                 name=f"badd{pc}")
                nc.vector.tensor_sub(badd[:], be, tmpb[:])
                yo = work.tile([m, L], F32, tag="yo", bufs=1,
                               name=f"yo{pc}")
                nc.vector.tensor_scalar(
                    yo[:], y[:], amul[:], badd[:], op0=ALU.mult, op1=ALU.add
                )
                nc.sync.dma_start(out=out_d[128 * pc:128 * pc + m, :], in_=yo[:])

    nc.compile()
    return nc


_NC = None


def _get_nc():
    global _NC
    if _NC is None:
        _NC = build_nc()
    return _NC


def make_in_maps(inputs, attention_mask, wq_w, wq_b, wk_w, wk_b, wv_w, wv_b,
                 wo_w, wo_b, gamma, beta):
    x = np.asarray(inputs, np.float32)
    am = np.asarray(attention_mask, np.int32)
    in_maps = []
    for c in range(NCORES):
        b, g = c // 4, c % 4
        hsl = slice(HF * g, HF * (g + 1))
        pcol = np.zeros((128, 16), np.float32)
        for j, vec in ((0, np.asarray(wq_b)[hsl]), (2, np.asarray(wk_b)[hsl]),
                       (4, np.asarray(wv_b)[hsl])):
            pcol[:, j] = vec[:128]
            pcol[:64, j + 1] = vec[128:]
        wob4 = np.asarray(wo_b, np.float32) / 4.0
        pcol[:, 6:12] = wob4.reshape(6, 128).T
        for j, vec in ((12, np.asarray(gamma)[hsl]), (14, np.asarray(beta)[hsl])):
            pcol[:, j] = vec[:128]
            pcol[:64, j + 1] = vec[128:]
        prow = np.zeros((1, 960), np.float32)
        prow[0, :HF] = np.asarray(wv_b)[hsl]
        prow[0, HF:] = wob4
        in_maps.append({
            "x": np.ascontiguousarray(x[b]),
            "wq": np.ascontiguousarray(np.asarray(wq_w, np.float32)[:, hsl]),
            "wk": np.ascontiguousarray(np.asarray(wk_w, np.float32)[:, hsl]),
            "wv": np.ascontiguousarray(np.asarray(wv_w, np.float32)[:, hsl]),
            "wo_r": np.ascontiguousarray(np.asarray(wo_w, np.float32)[hsl, :]),
            "mask_i": np.ascontiguousarray(am[b][None, :]),
            "params_col": pcol,
            "params_row": prow,
        })
    return in_maps


def run(trace=False, **inputs):
    nc = _get_nc()
    in_maps = make_in_maps(**inputs)
    res = run_bass_kernel_spmd(nc, in_maps, core_ids=list(range(NCORES)),
                               trace=trace)
    out = np.zeros((B, L, HIDDEN), np.float32)
    for c in range(NCORES):
        b, g = c // 4, c % 4
        out[b, :, HF * g:HF * (g + 1)] = res.results[c]["out_t"].T
    return out, res


def kernel(**inputs):
    out, _ = run(trace=False, **inputs)
    return out


# revision 24
# speedup vs baseline: 1.9592x; 1.9592x over previous
"""Trainium2 Bass kernel for nn_MultiHeadAttention (B=2, L=2048, H=768, 12 heads).

Sharding (8 cores): core c -> batch b=c//4, heads 3*(c%4)..3*(c%4)+2.
Each core: QKV proj for its 3 heads, flash-style attention (scores^T layout,
key-mask folded into V', query-mask folded into 1/l), partial output
projection with wo rows (row-parallel) + x/4 residual, ReduceScatter(add)
over the 4 cores of its batch, then layernorm over the sequence dim on its
192-row hidden slice. Host assembles [2,2048,768] from 8 [192,2048] slices.

PSUM static budget (8 banks): tag s = 2 bufs x [128,1024] (4 banks, shared by
scores / transposes / projections), tag av = [65,1024] (2), tag rb = [64,1024]
(2).
"""

import sys

import ml_dtypes
import numpy as np

BFNP = ml_dtypes.bfloat16

sys.path.insert(0, "/opt/trn_rl_repo")

import concourse.bass as bass  # noqa: E402
import concourse.bacc as bacc  # noqa: E402
import concourse.mybir as mybir  # noqa: E402
from concourse import tile  # noqa: E402
from concourse.bass_utils import run_bass_kernel_spmd  # noqa: E402

F32 = mybir.dt.float32
BF16 = mybir.dt.bfloat16
I32 = mybir.dt.int32
AF = mybir.ActivationFunctionType
ALU = mybir.AluOpType

HIDDEN = 768
HEADS = 12
HD = 64
L = 2048
B = 2
NCORES = 8
HPC = 3          # heads per core
HF = HPC * HD    # 192 features per core
LT = L // 128    # 16 l-tiles
HC = HIDDEN // 128  # 6 hidden chunks
OSL = HIDDEN // 4   # 192 output-slice rows per core


def build_nc():
    nc = bacc.Bacc("TRN2", target_bir_lowering=False, debug=False,
                   num_devices=NCORES)

    x_d = nc.dram_tensor("x", [L, HIDDEN], F32, kind="ExternalInput")
    wq_d = nc.dram_tensor("wq", [HIDDEN, HF], BF16, kind="ExternalInput")
    wk_d = nc.dram_tensor("wk", [HIDDEN, HF], BF16, kind="ExternalInput")
    wv_d = nc.dram_tensor("wv", [HIDDEN, HF], BF16, kind="ExternalInput")
    wo_d = nc.dram_tensor("wo_r", [HF, HIDDEN], BF16, kind="ExternalInput")
    mask_d = nc.dram_tensor("mask_i", [1, L], I32, kind="ExternalInput")
    # params_col[128, 16]: cols 0,1=wq_b(192) 2,3=wk_b 4,5=wv_b 6..11=wo_b/4
    # (768), 12,13=gamma slice, 14,15=beta slice
    pcol_d = nc.dram_tensor("params_col", [128, 16], F32, kind="ExternalInput")
    # params_row[1, 960]: 0:192 wv_b, 192:960 wo_b/4
    prow_d = nc.dram_tensor("params_row", [1, 960], BF16, kind="ExternalInput")
    out_d = nc.dram_tensor("out_t", [OSL, L], F32, kind="ExternalOutput")

    partial_d = nc.dram_tensor("partial_acc", [HIDDEN, L], F32)
    rs_d = nc.dram_tensor("rs_out", [OSL * L], F32)

    with tile.TileContext(nc) as tc:
        with (
            tc.tile_pool(name="persist", bufs=1) as pers,
            tc.tile_pool(name="xin", bufs=3) as xin,
            tc.tile_pool(name="work", bufs=2) as work,
            tc.tile_pool(name="ps2", bufs=2, space=bass.MemorySpace.PSUM) as ps2,
            tc.tile_pool(name="pav", bufs=1, space=bass.MemorySpace.PSUM) as pav,
            tc.tile_pool(name="prb", bufs=1, space=bass.MemorySpace.PSUM) as prb,
            tc.tile_pool(name="pexp", bufs=3) as pexp,
        ):
            def ps_tile(shape, name):
                return ps2.tile(shape, F32, tag="s", name=name,
                                padded_shape=[128, 1024])

            # ---------- phase 0: constants ----------
            ident_i = pers.tile([128, 128], I32, tag="ident_i")
            nc.gpsimd.iota(ident_i[:], pattern=[[-1, 128]], base=0,
                           channel_multiplier=1)
            ident = pers.tile([128, 128], F32, tag="ident")
            nc.vector.tensor_scalar(
                ident[:], ident_i[:], 0, None, op0=ALU.is_equal
            )
            ones_row = pers.tile([1, 512], F32, tag="ones_row")
            nc.vector.memset(ones_row[:], 1.0)
            ones_bf = pers.tile([1, 512], BF16, tag="ones_bf")
            nc.vector.memset(ones_bf[:], 1.0)

            pcol = pers.tile([128, 16], F32, tag="pcol")
            nc.sync.dma_start(out=pcol[:], in_=pcol_d[:])
            prow = pers.tile([1, 960], BF16, tag="prow")
            nc.sync.dma_start(out=prow[:], in_=prow_d[:])

            mask_i = xin.tile([1, L], I32, tag="mask_i", bufs=1)
            nc.sync.dma_start(out=mask_i[:], in_=mask_d[:])
            mask_row = pers.tile([1, L], F32, tag="mask_row")
            nc.vector.tensor_copy(mask_row[:], mask_i[:])

            # mask columns [128, 16]: col t = mask[128t:128t+128]
            mask_cols = pers.tile([128, LT], F32, tag="mask_cols")
            for t in range(LT):
                mp = ps_tile([128, 1], f"mask_ps{t}")
                nc.tensor.matmul(
                    mp[:], mask_row[:, 128 * t:128 * (t + 1)], ones_row[:, 0:1]
                )
                nc.vector.tensor_copy(mask_cols[:, t:t + 1], mp[:])

            # query-mask broadcast over 64 partitions, built once
            mask_bc = pers.tile([64, L], BF16, tag="mask_bc")
            for i in range(2):
                mb = ps_tile([64, 1024], f"mb{i}")
                for j in range(2):
                    nc.tensor.matmul(
                        mb[:, 512 * j:512 * (j + 1)],
                        ones_row[:, 0:64],
                        mask_row[:, 1024 * i + 512 * j:1024 * i + 512 * (j + 1)],
                    )
                nc.vector.tensor_copy(mask_bc[:, 1024 * i:1024 * (i + 1)], mb[:])

            # weights loaded early; tiny PE "touch" matmuls absorb each DMA
            # lane wait so later matmuls stay under the 2-wait limit
            wq = pers.tile([128, HC, HF], BF16, tag="wq")
            wk = pers.tile([128, HC, HF], BF16, tag="wk")
            wv = pers.tile([128, HC, HF], BF16, tag="wv")
            for w_sb, w_d in ((wq, wq_d), (wk, wk_d), (wv, wv_d)):
                nc.sync.dma_start(
                    out=w_sb[:], in_=w_d[:].rearrange("(c p) m -> p c m", p=128)
                )
            wo_a = pers.tile([128, HIDDEN], BF16, tag="wo_a")
            wo_b_sb = pers.tile([64, HIDDEN], BF16, tag="wo_b")
            nc.sync.dma_start(out=wo_a[:], in_=wo_d[0:128, :])
            nc.sync.dma_start(out=wo_b_sb[:], in_=wo_d[128:192, :])
            touch_srcs = (wq[:, 0, 0:1], wk[:, 0, 0:1], wv[:, 0, 0:1],
                          wo_a[:, 0:1], wo_b_sb[:, 0:1], prow[:, 0:1])
            tch = pav.tile([1, 1], F32, tag="av", name="touch",
                           padded_shape=[65, 1024])
            for ti, tsr in enumerate(touch_srcs):
                nc.tensor.matmul(tch[:], tsr, tsr, start=(ti == 0),
                                 stop=(ti == len(touch_srcs) - 1),
                                 skip_group_check=True)
            tch_scr = work.tile([1, 1], F32, tag="tch_scr", bufs=1)
            nc.scalar.copy(tch_scr[:], tch[:])

            # ---------- phase 1: load x, build x^T ----------
            x_t = [pers.tile([128, L], BF16, tag=f"x_t{c}", name=f"x_t{c}")
                   for c in range(HC)]
            for lt in range(LT):
                xn = xin.tile([128, HIDDEN], F32, tag="x_nat")
                nc.gpsimd.dma_start(out=xn[:], in_=x_d[128 * lt:128 * (lt + 1), :])
                for c in range(HC):
                    tp = ps_tile([128, 128], f"tr_ps{lt}_{c}")
                    nc.tensor.transpose(tp[:], xn[:, 128 * c:128 * (c + 1)], ident[:])
                    nc.vector.tensor_copy(
                        x_t[c][:, 128 * lt:128 * (lt + 1)], tp[:]
                    )

            # ---------- phase 2: QKV projections ----------
            # q^T / k^T: [192, L] as a [128, L] + [64, L] pair
            q_a = pers.tile([128, L], BF16, tag="q_a")
            k_a = pers.tile([128, L], BF16, tag="k_a")
            q_b_t = pers.tile([64, L], BF16, tag="q_b")
            k_b_t = pers.tile([64, L], BF16, tag="k_b")
            q_b = q_b_t[:]
            k_b = k_b_t[:]
            for wi, (dst, w_sb, bcol) in enumerate((
                ((q_a[:], q_b), wq, 0),
                ((k_a[:], k_b), wk, 2),
            )):
                for fc in range(2):  # feature chunk: 0 -> 128 rows, 1 -> 64 rows
                    m = 128 if fc == 0 else 64
                    for half in range(2):
                        ps = ps_tile([m, 1024], f"qk_ps{wi}_{fc}_{half}")
                        for qt in range(2):
                            sl = slice(512 * qt, 512 * (qt + 1))
                            xsl = slice(1024 * half + 512 * qt,
                                        1024 * half + 512 * (qt + 1))
                            for c in range(HC):
                                nc.tensor.matmul(
                                    ps[:, sl],
                                    w_sb[:, c, 128 * fc:128 * fc + m],
                                    x_t[c][:, xsl],
                                    start=(c == 0),
                                    stop=(c == HC - 1),
                                )
                        nc.vector.tensor_scalar_add(
                            dst[fc][:, 1024 * half:1024 * (half + 1)], ps[:],
                            pcol[0:m, bcol + fc:bcol + fc + 1]
                        )

            # V' tiles: [128, 3*65] per l-tile; per head h cols 65h..65h+63 =
            # (x@wv + b)*mask, col 65h+64 = mask
            v_sb = [work.tile([128, 3 * 65], BF16, tag=f"v{lt}", name=f"v{lt}",
                              bufs=1)
                    for lt in range(LT)]
            for lt in range(LT):
                vp = ps_tile([128, HF], f"v_ps{lt}")
                for c in range(HC):
                    nc.tensor.matmul(
                        vp[:],
                        x_t[c][:, 128 * lt:128 * (lt + 1)],
                        wv[:, c, :],
                        start=(c == 0),
                        stop=False,
                    )
                # + wv_b broadcast over rows: ones_col^T (K=1) x bias row
                nc.tensor.matmul(
                    vp[:],
                    ones_bf[:, 0:128],
                    prow[:, 0:HF],
                    start=False,
                    stop=True,
                )
                for h in range(HPC):
                    nc.vector.tensor_scalar_mul(
                        v_sb[lt][:, 65 * h:65 * h + 64],
                        vp[:, 64 * h:64 * (h + 1)],
                        mask_cols[:, lt:lt + 1],
                    )
                    nc.vector.tensor_copy(
                        v_sb[lt][:, 65 * h + 64:65 * h + 65],
                        mask_cols[:, lt:lt + 1],
                    )

            # ---------- phase 3: attention ----------
            attn_a = pers.tile([128, L], BF16, tag="attn_a")  # heads 0,1
            attn_b = pers.tile([64, L], BF16, tag="attn_b")   # head 2
            sp_tiles = []
            for h in range(HPC):
                if h == 0:
                    q_ap, k_ap, o_ap = q_a[0:64, :], k_a[0:64, :], attn_a[0:64, :]
                elif h == 1:
                    q_ap, k_ap, o_ap = (q_a[64:128, :], k_a[64:128, :],
                                        attn_a[64:128, :])
                else:
                    q_ap, k_ap, o_ap = q_b, k_b, attn_b[:]
                for qh in range(2):  # 1024-wide query halves
                    q0 = 1024 * qh
                    av = pav.tile([65, 1024], F32, tag="av", name=f"av{h}_{qh}")
                    for kt in range(LT):
                        sp = ps_tile([128, 1024], f"s{h}_{qh}_{kt}")
                        sp_tiles.append(sp)
                        for qq in range(2):
                            nc.tensor.matmul(
                                sp[:, 512 * qq:512 * (qq + 1)],
                                k_ap[:, 128 * kt:128 * (kt + 1)],
                                q_ap[:, q0 + 512 * qq:q0 + 512 * (qq + 1)],
                            )
                        pexp_t = pexp.tile([128, 1024], BF16, tag="p",
                                           name=f"p{h}_{qh}_{kt}")
                        nc.scalar.activation(pexp_t[:], sp[:], AF.Exp, scale=0.125)
                        for qq in range(2):
                            nc.tensor.matmul(
                                av[:, 512 * qq:512 * (qq + 1)],
                                v_sb[kt][:, 65 * h:65 * (h + 1)],
                                pexp_t[:, 512 * qq:512 * (qq + 1)],
                                start=(kt == 0),
                                stop=(kt == LT - 1),
                            )
                    # normalize: (1/l) * query-mask, broadcast over 64 partitions
                    # av drains via ACT so its slot-WAR merges with the exp
                    # wait on the next accumulation group's first matmul
                    av_sb = work.tile([64, 1024], F32, tag="av_sb", bufs=1,
                                      name=f"avs{h}_{qh}")
                    nc.scalar.copy(av_sb[:], av[0:64, :])
                    l_sb = work.tile([1, 1024], F32, tag="l_sb", bufs=1,
                                     name=f"l{h}_{qh}")
                    nc.scalar.copy(l_sb[:], av[64:65, :])
                    rb = prb.tile([64, 1024], F32, tag="rb", name=f"rb{h}_{qh}")
                    for i in range(2):
                        nc.tensor.matmul(
                            rb[:, 512 * i:512 * (i + 1)],
                            ones_row[:, 0:64],
                            l_sb[:, 512 * i:512 * (i + 1)],
                        )
                    rb_sb = work.tile([64, 1024], F32, tag="rb_sb", bufs=1,
                                      name=f"rbs{h}_{qh}")
                    nc.vector.reciprocal(rb_sb[:], rb[:])
                    nc.vector.tensor_mul(
                        rb_sb[:], rb_sb[:], mask_bc[:, q0:q0 + 1024]
                    )
                    nc.vector.tensor_mul(
                        o_ap[:, q0:q0 + 1024], av_sb[:], rb_sb[:]
                    )

            # ---------- phase 4: output projection (row-parallel partial) ----
            sp_scr = work.tile([1, 1], F32, tag="sp_scr", bufs=2)
            for spt in sp_tiles[-2:]:
                scr = work.tile([1, 1], F32, tag="sp_scr", bufs=2,
                                name="sp_touch")
                nc.vector.tensor_copy(scr[:], spt[0:1, 0:1])
            for oc in range(HC):
                st = work.tile([128, L], F32, tag="stage", bufs=2,
                               name=f"st{oc}")
                for half in range(2):
                    po = ps_tile([128, 1024], f"po{oc}_{half}")
                    for qt in range(2):
                        sl = slice(512 * qt, 512 * (qt + 1))
                        asl = slice(1024 * half + 512 * qt,
                                    1024 * half + 512 * (qt + 1))
                        nc.tensor.matmul(
                            po[:, sl],
                            wo_a[:, 128 * oc:128 * (oc + 1)],
                            attn_a[:, asl],
                            start=True,
                            stop=False,
                        )
                        nc.tensor.matmul(
                            po[:, sl],
                            wo_b_sb[:, 128 * oc:128 * (oc + 1)],
                            attn_b[:, asl],
                            start=False,
                            stop=False,
                        )
                        # + wo_b/4 broadcast over columns
                        nc.tensor.matmul(
                            po[:, sl],
                            prow[:, HF + 128 * oc:HF + 128 * (oc + 1)],
                            ones_bf[:, 0:512],
                            start=False,
                            stop=True,
                        )
                    # stage = x/4 + partial_out
                    nc.vector.scalar_tensor_tensor(
                        st[:, 1024 * half:1024 * (half + 1)],
                        x_t[oc][:, 1024 * half:1024 * (half + 1)],
                        0.25, po[:], op0=ALU.mult, op1=ALU.add,
                    )
                nc.gpsimd.dma_start(
                    out=partial_d[128 * oc:128 * (oc + 1), :], in_=st[:]
                )

            # ---------- phase 5: reduce-scatter + layernorm over L ----------
            nc.gpsimd.collective_compute(
                "ReduceScatter",
                ALU.add,
                replica_groups=[[0, 1, 2, 3], [4, 5, 6, 7]],
                ins=[partial_d[:].opt()],
                outs=[rs_d[:].opt()],
            )
            rs_ap = rs_d[:].rearrange("(r l) -> r l", l=L)
            for pc, m in ((0, 128), (1, 64)):
                y = work.tile([m, L], F32, tag="y", bufs=1, name=f"y{pc}")
                nc.sync.dma_start(out=y[:], in_=rs_ap[128 * pc:128 * pc + m, :])
                bnst = work.tile([m, 4 * 6], F32, tag=f"bnst{pc}", bufs=1,
                                 name=f"bnst{pc}")
                for cch in range(4):
                    nc.vector.bn_stats(
                        bnst[:, 6 * cch:6 * (cch + 1)],
                        y[:, 512 * cch:512 * (cch + 1)],
                    )
                stats = work.tile([m, 2], F32, tag=f"stats{pc}", bufs=1,
                                  name=f"stats{pc}")
                nc.vector.bn_aggr(stats[:], bnst[:])
                std = work.tile([m, 1], F32, tag=f"std{pc}", bufs=1,
                                name=f"std{pc}")
                nc.scalar.activation(
                    std[:], stats[:, 1:2], AF.Sqrt, scale=float(L) / float(L - 1)
                )
                rstd = work.tile([m, 1], F32, tag=f"rstd{pc}", bufs=1,
                                 name=f"rstd{pc}")
                nc.vector.reciprocal(rstd[:], std[:])
                ga = pcol[0:m, 12 + pc:13 + pc]
                be = pcol[0:m, 14 + pc:15 + pc]
                amul = work.tile([m, 1], F32, tag=f"amul{pc}", bufs=1,
                                 name=f"amul{pc}")
                nc.vector.tensor_mul(amul[:], rstd[:], ga)
                tmpb = work.tile([m, 1], F32, tag=f"tmpb{pc}", bufs=1,
                                 name=f"tmpb{pc}")
                nc.vector.tensor_mul(tmpb[:], stats[:, 0:1], amul[:])
                badd = work.tile([m, 1], F32, tag=f"badd{pc}", bufs=1,
                                 name=f"badd{pc}")
                nc.vector.tensor_sub(badd[:], be, tmpb[:])
                yo = work.tile([m, L], F32, tag="yo", bufs=1,
                               name=f"yo{pc}")
                nc.vector.tensor_scalar(
                    yo[:], y[:], amul[:], badd[:], op0=ALU.mult, op1=ALU.add
                )
                nc.sync.dma_start(out=out_d[128 * pc:128 * pc + m, :], in_=yo[:])

    nc.compile()
    return nc


_NC = None


def _get_nc():
    global _NC
    if _NC is None:
        _NC = build_nc()
    return _NC


def make_in_maps(inputs, attention_mask, wq_w, wq_b, wk_w, wk_b, wv_w, wv_b,
                 wo_w, wo_b, gamma, beta):
    x = np.asarray(inputs, np.float32)
    am = np.asarray(attention_mask, np.int32)
    in_maps = []
    for c in range(NCORES):
        b, g = c // 4, c % 4
        hsl = slice(HF * g, HF * (g + 1))
        pcol = np.zeros((128, 16), np.float32)
        for j, vec in ((0, np.asarray(wq_b)[hsl]), (2, np.asarray(wk_b)[hsl]),
                       (4, np.asarray(wv_b)[hsl])):
            pcol[:, j] = vec[:128]
            pcol[:64, j + 1] = vec[128:]
        wob4 = np.asarray(wo_b, np.float32) / 4.0
        pcol[:, 6:12] = wob4.reshape(6, 128).T
        for j, vec in ((12, np.asarray(gamma)[hsl]), (14, np.asarray(beta)[hsl])):
            pcol[:, j] = vec[:128]
            pcol[:64, j + 1] = vec[128:]
        prow = np.zeros((1, 960), BFNP)
        prow[0, :HF] = np.asarray(wv_b)[hsl]
        prow[0, HF:] = wob4
        in_maps.append({
            "x": np.ascontiguousarray(x[b]),
            "wq": np.ascontiguousarray(np.asarray(wq_w, np.float32)[:, hsl].astype(BFNP)),
            "wk": np.ascontiguousarray(np.asarray(wk_w, np.float32)[:, hsl].astype(BFNP)),
            "wv": np.ascontiguousarray(np.asarray(wv_w, np.float32)[:, hsl].astype(BFNP)),
            "wo_r": np.ascontiguousarray(np.asarray(wo_w, np.float32)[hsl, :].astype(BFNP)),
            "mask_i": np.ascontiguousarray(am[b][None, :]),
            "params_col": pcol,
            "params_row": prow,
        })
    return in_maps


def run(trace=False, **inputs):
    nc = _get_nc()
    in_maps = make_in_maps(**inputs)
    res = run_bass_kernel_spmd(nc, in_maps, core_ids=list(range(NCORES)),
                               trace=trace)
    out = np.zeros((B, L, HIDDEN), np.float32)
    for c in range(NCORES):
        b, g = c // 4, c % 4
        out[b, :, HF * g:HF * (g + 1)] = res.results[c]["out_t"].T
    return out, res


def kernel(**inputs):
    out, _ = run(trace=False, **inputs)
    return out


# revision 26
# speedup vs baseline: 2.0353x; 1.0388x over previous
"""Trainium2 Bass kernel for nn_MultiHeadAttention (B=2, L=2048, H=768, 12 heads).

Sharding (8 cores): core c -> batch b=c//4, heads 3*(c%4)..3*(c%4)+2.
Each core: QKV proj for its 3 heads, flash-style attention (scores^T layout,
key-mask folded into V', query-mask folded into 1/l), partial output
projection with wo rows (row-parallel) + x/4 residual, ReduceScatter(add)
over the 4 cores of its batch, then layernorm over the sequence dim on its
192-row hidden slice. Host assembles [2,2048,768] from 8 [192,2048] slices.

PSUM static budget (8 banks): tag s = 2 bufs x [128,1024] (4 banks, shared by
scores / transposes / projections), tag av = [65,1024] (2), tag rb = [64,1024]
(2).
"""

import sys

import ml_dtypes
import numpy as np

BFNP = ml_dtypes.bfloat16

sys.path.insert(0, "/opt/trn_rl_repo")

import concourse.bass as bass  # noqa: E402
import concourse.bacc as bacc  # noqa: E402
import concourse.mybir as mybir  # noqa: E402
from concourse import tile  # noqa: E402
from concourse.bass_utils import run_bass_kernel_spmd  # noqa: E402

F32 = mybir.dt.float32
BF16 = mybir.dt.bfloat16
I32 = mybir.dt.int32
AF = mybir.ActivationFunctionType
ALU = mybir.AluOpType

HIDDEN = 768
HEADS = 12
HD = 64
L = 2048
B = 2
NCORES = 8
HPC = 3          # heads per core
HF = HPC * HD    # 192 features per core
LT = L // 128    # 16 l-tiles
HC = HIDDEN // 128  # 6 hidden chunks
OSL = HIDDEN // 4   # 192 output-slice rows per core


def build_nc():
    nc = bacc.Bacc("TRN2", target_bir_lowering=False, debug=False,
                   num_devices=NCORES)

    x_d = nc.dram_tensor("x", [L, HIDDEN], F32, kind="ExternalInput")
    wq_d = nc.dram_tensor("wq", [HIDDEN, HF], BF16, kind="ExternalInput")
    wk_d = nc.dram_tensor("wk", [HIDDEN, HF], BF16, kind="ExternalInput")
    wv_d = nc.dram_tensor("wv", [HIDDEN, HF], BF16, kind="ExternalInput")
    wo_d = nc.dram_tensor("wo_r", [HF, HIDDEN], BF16, kind="ExternalInput")
    mask_d = nc.dram_tensor("mask_i", [1, L], I32, kind="ExternalInput")
    # params_col[128, 16]: cols 0,1=wq_b(192) 2,3=wk_b 4,5=wv_b 6..11=wo_b/4
    # (768), 12,13=gamma slice, 14,15=beta slice
    pcol_d = nc.dram_tensor("params_col", [128, 16], F32, kind="ExternalInput")
    # params_row[1, 960]: 0:192 wv_b, 192:960 wo_b/4
    prow_d = nc.dram_tensor("params_row", [1, 960], BF16, kind="ExternalInput")
    xr_d = nc.dram_tensor("xr", [L, OSL], F32, kind="ExternalInput")
    out_d = nc.dram_tensor("out_t", [OSL, L], F32, kind="ExternalOutput")

    partial_d = nc.dram_tensor("partial_acc", [HIDDEN, L], F32)
    rs_d = nc.dram_tensor("rs_out", [OSL * L], F32)

    with tile.TileContext(nc) as tc:
        with (
            tc.tile_pool(name="persist", bufs=1) as pers,
            tc.tile_pool(name="xin", bufs=3) as xin,
            tc.tile_pool(name="work", bufs=2) as work,
            tc.tile_pool(name="ps2", bufs=2, space=bass.MemorySpace.PSUM) as ps2,
            tc.tile_pool(name="pav", bufs=1, space=bass.MemorySpace.PSUM) as pav,
            tc.tile_pool(name="prb", bufs=1, space=bass.MemorySpace.PSUM) as prb,
            tc.tile_pool(name="pexp", bufs=3) as pexp,
        ):
            def ps_tile(shape, name):
                return ps2.tile(shape, F32, tag="s", name=name,
                                padded_shape=[128, 1024])

            # ---------- phase 0: constants ----------
            ident_i = pers.tile([128, 128], I32, tag="ident_i")
            nc.gpsimd.iota(ident_i[:], pattern=[[-1, 128]], base=0,
                           channel_multiplier=1)
            ident = pers.tile([128, 128], F32, tag="ident")
            nc.vector.tensor_scalar(
                ident[:], ident_i[:], 0, None, op0=ALU.is_equal
            )
            ones_row = pers.tile([1, 512], F32, tag="ones_row")
            nc.vector.memset(ones_row[:], 1.0)
            ones_bf = pers.tile([1, 512], BF16, tag="ones_bf")
            nc.vector.memset(ones_bf[:], 1.0)

            pcol = pers.tile([128, 16], F32, tag="pcol")
            nc.sync.dma_start(out=pcol[:], in_=pcol_d[:])
            prow = pers.tile([1, 960], BF16, tag="prow")
            nc.sync.dma_start(out=prow[:], in_=prow_d[:])

            mask_i = xin.tile([1, L], I32, tag="mask_i", bufs=1)
            nc.sync.dma_start(out=mask_i[:], in_=mask_d[:])
            mask_row = pers.tile([1, L], F32, tag="mask_row")
            nc.vector.tensor_copy(mask_row[:], mask_i[:])

            # mask columns [128, 16]: col t = mask[128t:128t+128]
            mask_cols = pers.tile([128, LT], F32, tag="mask_cols")
            for t in range(LT):
                mp = ps_tile([128, 1], f"mask_ps{t}")
                nc.tensor.matmul(
                    mp[:], mask_row[:, 128 * t:128 * (t + 1)], ones_row[:, 0:1]
                )
                nc.vector.tensor_copy(mask_cols[:, t:t + 1], mp[:])

            # query-mask broadcast over 64 partitions, built once
            mask_bc = pers.tile([64, L], BF16, tag="mask_bc")
            for i in range(2):
                mb = ps_tile([64, 1024], f"mb{i}")
                for j in range(2):
                    nc.tensor.matmul(
                        mb[:, 512 * j:512 * (j + 1)],
                        ones_row[:, 0:64],
                        mask_row[:, 1024 * i + 512 * j:1024 * i + 512 * (j + 1)],
                    )
                nc.vector.tensor_copy(mask_bc[:, 1024 * i:1024 * (i + 1)], mb[:])

            # weights loaded early; tiny PE "touch" matmuls absorb each DMA
            # lane wait so later matmuls stay under the 2-wait limit
            wq = pers.tile([128, HC, HF], BF16, tag="wq")
            wk = pers.tile([128, HC, HF], BF16, tag="wk")
            wv = pers.tile([128, HC, HF], BF16, tag="wv")
            for w_sb, w_d in ((wq, wq_d), (wk, wk_d), (wv, wv_d)):
                nc.sync.dma_start(
                    out=w_sb[:], in_=w_d[:].rearrange("(c p) m -> p c m", p=128)
                )
            wo_a = pers.tile([128, HIDDEN], BF16, tag="wo_a")
            wo_b_sb = pers.tile([64, HIDDEN], BF16, tag="wo_b")
            nc.sync.dma_start(out=wo_a[:], in_=wo_d[0:128, :])
            nc.sync.dma_start(out=wo_b_sb[:], in_=wo_d[128:192, :])
            touch_srcs = (wq[:, 0, 0:1], wk[:, 0, 0:1], wv[:, 0, 0:1],
                          wo_a[:, 0:1], wo_b_sb[:, 0:1], prow[:, 0:1])
            tch = pav.tile([1, 1], F32, tag="av", name="touch",
                           padded_shape=[65, 1024])
            for ti, tsr in enumerate(touch_srcs):
                nc.tensor.matmul(tch[:], tsr, tsr, start=(ti == 0),
                                 stop=(ti == len(touch_srcs) - 1),
                                 skip_group_check=True)
            tch_scr = work.tile([1, 1], F32, tag="tch_scr", bufs=1)
            nc.scalar.copy(tch_scr[:], tch[:])

            # ---------- phase 1: load x, build x^T ----------
            x_t = [pers.tile([128, L], BF16, tag=f"x_t{c}", name=f"x_t{c}")
                   for c in range(HC)]
            for lt in range(LT):
                xn = xin.tile([128, HIDDEN], F32, tag="x_nat")
                nc.gpsimd.dma_start(out=xn[:], in_=x_d[128 * lt:128 * (lt + 1), :])
                for c in range(HC):
                    tp = ps_tile([128, 128], f"tr_ps{lt}_{c}")
                    nc.tensor.transpose(tp[:], xn[:, 128 * c:128 * (c + 1)], ident[:])
                    nc.vector.tensor_copy(
                        x_t[c][:, 128 * lt:128 * (lt + 1)], tp[:]
                    )

            # ---------- phase 1.5: residual slice x^T (fp32) ----------
            xr_t_a = pers.tile([128, L], F32, tag="xr_t_a")
            xr_t_b = pers.tile([64, L], F32, tag="xr_t_b")
            for lt in range(LT):
                xrn = xin.tile([128, OSL], F32, tag="xr_nat")
                nc.gpsimd.dma_start(out=xrn[:],
                                    in_=xr_d[128 * lt:128 * (lt + 1), :])
                tp = ps_tile([128, 128], f"xr_ps{lt}_0")
                nc.tensor.transpose(tp[:], xrn[:, 0:128], ident[:])
                nc.vector.tensor_copy(xr_t_a[:, 128 * lt:128 * (lt + 1)], tp[:])
                tp2 = ps_tile([64, 128], f"xr_ps{lt}_1")
                nc.tensor.transpose(tp2[:], xrn[:, 128:192], ident[:])
                nc.vector.tensor_copy(xr_t_b[:, 128 * lt:128 * (lt + 1)], tp2[:])

            # ---------- phase 2: QKV projections ----------
            # q^T / k^T: [192, L] as a [128, L] + [64, L] pair
            q_a = pers.tile([128, L], BF16, tag="q_a")
            k_a = pers.tile([128, L], BF16, tag="k_a")
            q_b_t = pers.tile([64, L], BF16, tag="q_b")
            k_b_t = pers.tile([64, L], BF16, tag="k_b")
            q_b = q_b_t[:]
            k_b = k_b_t[:]
            for wi, (dst, w_sb, bcol) in enumerate((
                ((q_a[:], q_b), wq, 0),
                ((k_a[:], k_b), wk, 2),
            )):
                for fc in range(2):  # feature chunk: 0 -> 128 rows, 1 -> 64 rows
                    m = 128 if fc == 0 else 64
                    for half in range(2):
                        ps = ps_tile([m, 1024], f"qk_ps{wi}_{fc}_{half}")
                        for qt in range(2):
                            sl = slice(512 * qt, 512 * (qt + 1))
                            xsl = slice(1024 * half + 512 * qt,
                                        1024 * half + 512 * (qt + 1))
                            for c in range(HC):
                                nc.tensor.matmul(
                                    ps[:, sl],
                                    w_sb[:, c, 128 * fc:128 * fc + m],
                                    x_t[c][:, xsl],
                                    start=(c == 0),
                                    stop=(c == HC - 1),
                                )
                        nc.vector.tensor_scalar_add(
                            dst[fc][:, 1024 * half:1024 * (half + 1)], ps[:],
                            pcol[0:m, bcol + fc:bcol + fc + 1]
                        )

            # V' tiles: [128, 3*65] per l-tile; per head h cols 65h..65h+63 =
            # (x@wv + b)*mask, col 65h+64 = mask
            v_sb = [work.tile([128, 3 * 65], BF16, tag=f"v{lt}", name=f"v{lt}",
                              bufs=1)
                    for lt in range(LT)]
            for lt in range(LT):
                vp = ps_tile([128, HF], f"v_ps{lt}")
                for c in range(HC):
                    nc.tensor.matmul(
                        vp[:],
                        x_t[c][:, 128 * lt:128 * (lt + 1)],
                        wv[:, c, :],
                        start=(c == 0),
                        stop=False,
                    )
                # + wv_b broadcast over rows: ones_col^T (K=1) x bias row
                nc.tensor.matmul(
                    vp[:],
                    ones_bf[:, 0:128],
                    prow[:, 0:HF],
                    start=False,
                    stop=True,
                )
                for h in range(HPC):
                    nc.vector.tensor_scalar_mul(
                        v_sb[lt][:, 65 * h:65 * h + 64],
                        vp[:, 64 * h:64 * (h + 1)],
                        mask_cols[:, lt:lt + 1],
                    )
                    nc.vector.tensor_copy(
                        v_sb[lt][:, 65 * h + 64:65 * h + 65],
                        mask_cols[:, lt:lt + 1],
                    )

            # ---------- phase 3: attention ----------
            attn_a = pers.tile([128, L], BF16, tag="attn_a")  # heads 0,1
            attn_b = pers.tile([64, L], BF16, tag="attn_b")   # head 2
            sp_tiles = []
            for h in range(HPC):
                if h == 0:
                    q_ap, k_ap, o_ap = q_a[0:64, :], k_a[0:64, :], attn_a[0:64, :]
                elif h == 1:
                    q_ap, k_ap, o_ap = (q_a[64:128, :], k_a[64:128, :],
                                        attn_a[64:128, :])
                else:
                    q_ap, k_ap, o_ap = q_b, k_b, attn_b[:]
                for qh in range(2):  # 1024-wide query halves
                    q0 = 1024 * qh
                    av = pav.tile([65, 1024], F32, tag="av", name=f"av{h}_{qh}")
                    for kt in range(LT):
                        sp = ps_tile([128, 1024], f"s{h}_{qh}_{kt}")
                        sp_tiles.append(sp)
                        for qq in range(2):
                            nc.tensor.matmul(
                                sp[:, 512 * qq:512 * (qq + 1)],
                                k_ap[:, 128 * kt:128 * (kt + 1)],
                                q_ap[:, q0 + 512 * qq:q0 + 512 * (qq + 1)],
                            )
                        pexp_t = pexp.tile([128, 1024], BF16, tag="p",
                                           name=f"p{h}_{qh}_{kt}")
                        nc.scalar.activation(pexp_t[:], sp[:], AF.Exp, scale=0.125)
                        for qq in range(2):
                            nc.tensor.matmul(
                                av[:, 512 * qq:512 * (qq + 1)],
                                v_sb[kt][:, 65 * h:65 * (h + 1)],
                                pexp_t[:, 512 * qq:512 * (qq + 1)],
                                start=(kt == 0),
                                stop=(kt == LT - 1),
                            )
                    # normalize: (1/l) * query-mask, broadcast over 64 partitions
                    # av drains via ACT so its slot-WAR merges with the exp
                    # wait on the next accumulation group's first matmul
                    av_sb = work.tile([64, 1024], F32, tag="av_sb", bufs=1,
                                      name=f"avs{h}_{qh}")
                    nc.scalar.copy(av_sb[:], av[0:64, :])
                    l_sb = work.tile([1, 1024], BF16, tag="l_sb", bufs=1,
                                     name=f"l{h}_{qh}")
                    nc.scalar.copy(l_sb[:], av[64:65, :])
                    rb = prb.tile([64, 1024], F32, tag="rb", name=f"rb{h}_{qh}")
                    for i in range(2):
                        nc.tensor.matmul(
                            rb[:, 512 * i:512 * (i + 1)],
                            ones_bf[:, 0:64],
                            l_sb[:, 512 * i:512 * (i + 1)],
                        )
                    rb_sb = work.tile([64, 1024], F32, tag="rb_sb", bufs=1,
                                      name=f"rbs{h}_{qh}")
                    nc.vector.reciprocal(rb_sb[:], rb[:])
                    nc.vector.tensor_mul(
                        rb_sb[:], rb_sb[:], mask_bc[:, q0:q0 + 1024]
                    )
                    nc.vector.tensor_mul(
                        o_ap[:, q0:q0 + 1024], av_sb[:], rb_sb[:]
                    )

            # ---------- phase 4: output projection (row-parallel partial) ----
            sp_scr = work.tile([1, 1], F32, tag="sp_scr", bufs=2)
            for spt in sp_tiles[-2:]:
                scr = work.tile([1, 1], F32, tag="sp_scr", bufs=2,
                                name="sp_touch")
                nc.vector.tensor_copy(scr[:], spt[0:1, 0:1])
            for oc in range(HC):
                st = work.tile([128, L], F32, tag="stage", bufs=2,
                               name=f"st{oc}")
                for half in range(2):
                    po = ps_tile([128, 1024], f"po{oc}_{half}")
                    for qt in range(2):
                        sl = slice(512 * qt, 512 * (qt + 1))
                        asl = slice(1024 * half + 512 * qt,
                                    1024 * half + 512 * (qt + 1))
                        nc.tensor.matmul(
                            po[:, sl],
                            wo_a[:, 128 * oc:128 * (oc + 1)],
                            attn_a[:, asl],
                            start=True,
                            stop=False,
                        )
                        nc.tensor.matmul(
                            po[:, sl],
                            wo_b_sb[:, 128 * oc:128 * (oc + 1)],
                            attn_b[:, asl],
                            start=False,
                            stop=False,
                        )
                        # + wo_b/4 broadcast over columns
                        nc.tensor.matmul(
                            po[:, sl],
                            prow[:, HF + 128 * oc:HF + 128 * (oc + 1)],
                            ones_bf[:, 0:512],
                            start=False,
                            stop=True,
                        )
                    nc.vector.tensor_copy(
                        st[:, 1024 * half:1024 * (half + 1)], po[:]
                    )
                nc.gpsimd.dma_start(
                    out=partial_d[128 * oc:128 * (oc + 1), :], in_=st[:]
                )

            # ---------- phase 5: reduce-scatter + layernorm over L ----------
            nc.gpsimd.collective_compute(
                "ReduceScatter",
                ALU.add,
                replica_groups=[[0, 1, 2, 3], [4, 5, 6, 7]],
                ins=[partial_d[:].opt()],
                outs=[rs_d[:].opt()],
            )
            rs_ap = rs_d[:].rearrange("(r l) -> r l", l=L)
            for pc, m in ((0, 128), (1, 64)):
                yb = work.tile([m, L], F32, tag="yb", bufs=1, name=f"yb{pc}")
                nc.sync.dma_start(out=yb[:], in_=rs_ap[128 * pc:128 * pc + m, :])
                xr_ap = xr_t_a[:] if pc == 0 else xr_t_b[:]
                y = work.tile([m, L], F32, tag="y", bufs=1, name=f"y{pc}")
                nc.vector.tensor_add(y[:], xr_ap, yb[:])
                bnst = work.tile([m, 4 * 6], F32, tag=f"bnst{pc}", bufs=1,
                                 name=f"bnst{pc}")
                for cch in range(4):
                    nc.vector.bn_stats(
                        bnst[:, 6 * cch:6 * (cch + 1)],
                        y[:, 512 * cch:512 * (cch + 1)],
                    )
                stats = work.tile([m, 2], F32, tag=f"stats{pc}", bufs=1,
                                  name=f"stats{pc}")
                nc.vector.bn_aggr(stats[:], bnst[:])
                std = work.tile([m, 1], F32, tag=f"std{pc}", bufs=1,
                                name=f"std{pc}")
                nc.scalar.activation(
                    std[:], stats[:, 1:2], AF.Sqrt, scale=float(L) / float(L - 1)
                )
                rstd = work.tile([m, 1], F32, tag=f"rstd{pc}", bufs=1,
                                 name=f"rstd{pc}")
                nc.vector.reciprocal(rstd[:], std[:])
                ga = pcol[0:m, 12 + pc:13 + pc]
                be = pcol[0:m, 14 + pc:15 + pc]
                amul = work.tile([m, 1], F32, tag=f"amul{pc}", bufs=1,
                                 name=f"amul{pc}")
                nc.vector.tensor_mul(amul[:], rstd[:], ga)
                tmpb = work.tile([m, 1], F32, tag=f"tmpb{pc}", bufs=1,
                                 name=f"tmpb{pc}")
                nc.vector.tensor_mul(tmpb[:], stats[:, 0:1], amul[:])
                badd = work.tile([m, 1], F32, tag=f"badd{pc}", bufs=1,
                                 name=f"badd{pc}")
                nc.vector.tensor_sub(badd[:], be, tmpb[:])
                yo = work.tile([m, L], F32, tag="yo", bufs=1,
                               name=f"yo{pc}")
                nc.vector.tensor_scalar(
                    yo[:], y[:], amul[:], badd[:], op0=ALU.mult, op1=ALU.add
                )
                nc.sync.dma_start(out=out_d[128 * pc:128 * pc + m, :], in_=yo[:])

    nc.compile()
    return nc


_NC = None


def _get_nc():
    global _NC
    if _NC is None:
        _NC = build_nc()
    return _NC


def make_in_maps(inputs, attention_mask, wq_w, wq_b, wk_w, wk_b, wv_w, wv_b,
                 wo_w, wo_b, gamma, beta):
    x = np.asarray(inputs, np.float32)
    am = np.asarray(attention_mask, np.int32)
    in_maps = []
    for c in range(NCORES):
        b, g = c // 4, c % 4
        hsl = slice(HF * g, HF * (g + 1))
        pcol = np.zeros((128, 16), np.float32)
        for j, vec in ((0, np.asarray(wq_b)[hsl]), (2, np.asarray(wk_b)[hsl]),
                       (4, np.asarray(wv_b)[hsl])):
            pcol[:, j] = vec[:128]
            pcol[:64, j + 1] = vec[128:]
        wob4 = np.asarray(wo_b, np.float32) / 4.0
        pcol[:, 6:12] = wob4.reshape(6, 128).T
        for j, vec in ((12, np.asarray(gamma)[hsl]), (14, np.asarray(beta)[hsl])):
            pcol[:, j] = vec[:128]
            pcol[:64, j + 1] = vec[128:]
        prow = np.zeros((1, 960), BFNP)
        prow[0, :HF] = np.asarray(wv_b)[hsl]
        prow[0, HF:] = wob4
        in_maps.append({
            "x": np.ascontiguousarray(x[b]),
            "xr": np.ascontiguousarray(x[b][:, hsl]),
            "wq": np.ascontiguousarray(np.asarray(wq_w, np.float32)[:, hsl].astype(BFNP)),
            "wk": np.ascontiguousarray(np.asarray(wk_w, np.float32)[:, hsl].astype(BFNP)),
            "wv": np.ascontiguousarray(np.asarray(wv_w, np.float32)[:, hsl].astype(BFNP)),
            "wo_r": np.ascontiguousarray(np.asarray(wo_w, np.float32)[hsl, :].astype(BFNP)),
            "mask_i": np.ascontiguousarray(am[b][None, :]),
            "params_col": pcol,
            "params_row": prow,
        })
    return in_maps


def run(trace=False, **inputs):
    nc = _get_nc()
    in_maps = make_in_maps(**inputs)
    res = run_bass_kernel_spmd(nc, in_maps, core_ids=list(range(NCORES)),
                               trace=trace)
    out = np.zeros((B, L, HIDDEN), np.float32)
    for c in range(NCORES):
        b, g = c // 4, c % 4
        out[b, :, HF * g:HF * (g + 1)] = res.results[c]["out_t"].T
    return out, res


def kernel(**inputs):
    out, _ = run(trace=False, **inputs)
    return out


# revision 28
# speedup vs baseline: 2.1316x; 1.0473x over previous
"""Trainium2 Bass kernel for nn_MultiHeadAttention (B=2, L=2048, H=768, 12 heads).

Sharding (8 cores): core c -> batch b=c//4, heads 3*(c%4)..3*(c%4)+2.
Each core: QKV proj for its 3 heads, flash-style attention (scores^T layout,
key-mask folded into V', query-mask folded into 1/l), partial output
projection with wo rows (row-parallel) + x/4 residual, ReduceScatter(add)
over the 4 cores of its batch, then layernorm over the sequence dim on its
192-row hidden slice. Host assembles [2,2048,768] from 8 [192,2048] slices.

PSUM static budget (8 banks): tag s = 2 bufs x [128,1024] (4 banks, shared by
scores / transposes / projections), tag av = [65,1024] (2), tag rb = [64,1024]
(2).
"""

import sys

import ml_dtypes
import numpy as np

BFNP = ml_dtypes.bfloat16

sys.path.insert(0, "/opt/trn_rl_repo")

import concourse.bass as bass  # noqa: E402
import concourse.bacc as bacc  # noqa: E402
import concourse.mybir as mybir  # noqa: E402
from concourse import tile  # noqa: E402
from concourse.bass_utils import run_bass_kernel_spmd  # noqa: E402

F32 = mybir.dt.float32
BF16 = mybir.dt.bfloat16
I32 = mybir.dt.int32
AF = mybir.ActivationFunctionType
ALU = mybir.AluOpType

HIDDEN = 768
HEADS = 12
HD = 64
L = 2048
B = 2
NCORES = 8
HPC = 3          # heads per core
HF = HPC * HD    # 192 features per core
LT = L // 128    # 16 l-tiles
HC = HIDDEN // 128  # 6 hidden chunks
OSL = HIDDEN // 4   # 192 output-slice rows per core


def build_nc():
    nc = bacc.Bacc("TRN2", target_bir_lowering=False, debug=False,
                   num_devices=NCORES)

    x_d = nc.dram_tensor("x", [L, HIDDEN], F32, kind="ExternalInput")
    wq_d = nc.dram_tensor("wq", [HIDDEN, HF], BF16, kind="ExternalInput")
    wk_d = nc.dram_tensor("wk", [HIDDEN, HF], BF16, kind="ExternalInput")
    wv_d = nc.dram_tensor("wv", [HIDDEN, HF], BF16, kind="ExternalInput")
    wo_d = nc.dram_tensor("wo_r", [HF, HIDDEN], BF16, kind="ExternalInput")
    mask_d = nc.dram_tensor("mask_i", [1, L], I32, kind="ExternalInput")
    # params_col[128, 16]: cols 0,1=wq_b(192) 2,3=wk_b 4,5=wv_b 6..11=wo_b/4
    # (768), 12,13=gamma slice, 14,15=beta slice
    pcol_d = nc.dram_tensor("params_col", [128, 16], F32, kind="ExternalInput")
    # params_row[1, 960]: 0:192 wv_b, 192:960 wo_b/4
    prow_d = nc.dram_tensor("params_row", [1, 960], BF16, kind="ExternalInput")
    xr_d = nc.dram_tensor("xr", [L, OSL], F32, kind="ExternalInput")
    out_d = nc.dram_tensor("out_t", [OSL, L], F32, kind="ExternalOutput")

    partial_d = nc.dram_tensor("partial_acc", [HIDDEN, L], F32)
    rs_d = nc.dram_tensor("rs_out", [OSL * L], F32)

    with tile.TileContext(nc) as tc:
        with (
            tc.tile_pool(name="persist", bufs=1) as pers,
            tc.tile_pool(name="xin", bufs=3) as xin,
            tc.tile_pool(name="work", bufs=2) as work,
            tc.tile_pool(name="ps2", bufs=2, space=bass.MemorySpace.PSUM) as ps2,
            tc.tile_pool(name="pav", bufs=2, space=bass.MemorySpace.PSUM) as pav,
            tc.tile_pool(name="pexp", bufs=3) as pexp,
        ):
            def ps_tile(shape, name):
                return ps2.tile(shape, F32, tag="s", name=name,
                                padded_shape=[128, 1024])

            # ---------- phase 0: constants ----------
            ident_i = pers.tile([128, 128], I32, tag="ident_i")
            nc.gpsimd.iota(ident_i[:], pattern=[[-1, 128]], base=0,
                           channel_multiplier=1)
            ident = pers.tile([128, 128], F32, tag="ident")
            nc.vector.tensor_scalar(
                ident[:], ident_i[:], 0, None, op0=ALU.is_equal
            )
            ones_row = pers.tile([1, 512], F32, tag="ones_row")
            nc.vector.memset(ones_row[:], 1.0)
            ones_bf = pers.tile([1, 512], BF16, tag="ones_bf")
            nc.vector.memset(ones_bf[:], 1.0)

            pcol = pers.tile([128, 16], F32, tag="pcol")
            nc.sync.dma_start(out=pcol[:], in_=pcol_d[:])
            prow = pers.tile([1, 960], BF16, tag="prow")
            nc.sync.dma_start(out=prow[:], in_=prow_d[:])

            mask_i = xin.tile([1, L], I32, tag="mask_i", bufs=1)
            nc.sync.dma_start(out=mask_i[:], in_=mask_d[:])
            mask_row = pers.tile([1, L], F32, tag="mask_row")
            nc.vector.tensor_copy(mask_row[:], mask_i[:])

            # mask columns [128, 16]: col t = mask[128t:128t+128]
            mask_cols = pers.tile([128, LT], F32, tag="mask_cols")
            for t in range(LT):
                mp = ps_tile([128, 1], f"mask_ps{t}")
                nc.tensor.matmul(
                    mp[:], mask_row[:, 128 * t:128 * (t + 1)], ones_row[:, 0:1]
                )
                nc.vector.tensor_copy(mask_cols[:, t:t + 1], mp[:])

            # query-mask broadcast over 64 partitions, built once
            mask_bc = pers.tile([64, L], BF16, tag="mask_bc")
            for i in range(2):
                mb = ps_tile([64, 1024], f"mb{i}")
                for j in range(2):
                    nc.tensor.matmul(
                        mb[:, 512 * j:512 * (j + 1)],
                        ones_row[:, 0:64],
                        mask_row[:, 1024 * i + 512 * j:1024 * i + 512 * (j + 1)],
                    )
                nc.vector.tensor_copy(mask_bc[:, 1024 * i:1024 * (i + 1)], mb[:])

            # weights loaded early; tiny PE "touch" matmuls absorb each DMA
            # lane wait so later matmuls stay under the 2-wait limit
            wq = pers.tile([128, HC, HF], BF16, tag="wq")
            wk = pers.tile([128, HC, HF], BF16, tag="wk")
            wv = pers.tile([128, HC, HF], BF16, tag="wv")
            for w_sb, w_d in ((wq, wq_d), (wk, wk_d), (wv, wv_d)):
                nc.sync.dma_start(
                    out=w_sb[:], in_=w_d[:].rearrange("(c p) m -> p c m", p=128)
                )
            wo_a = pers.tile([128, HIDDEN], BF16, tag="wo_a")
            wo_b_sb = pers.tile([64, HIDDEN], BF16, tag="wo_b")
            nc.sync.dma_start(out=wo_a[:], in_=wo_d[0:128, :])
            nc.sync.dma_start(out=wo_b_sb[:], in_=wo_d[128:192, :])
            touch_srcs = (wq[:, 0, 0:1], wk[:, 0, 0:1], wv[:, 0, 0:1],
                          wo_a[:, 0:1], wo_b_sb[:, 0:1], prow[:, 0:1])
            tch = pav.tile([1, 1], F32, tag="av", name="touch",
                           padded_shape=[65, 1024])
            for ti, tsr in enumerate(touch_srcs):
                nc.tensor.matmul(tch[:], tsr, tsr, start=(ti == 0),
                                 stop=(ti == len(touch_srcs) - 1),
                                 skip_group_check=True)
            tch_scr = work.tile([1, 1], F32, tag="tch_scr", bufs=1)
            nc.scalar.copy(tch_scr[:], tch[:])

            # ---------- phase 1: load x, build x^T ----------
            x_t = [pers.tile([128, L], BF16, tag=f"x_t{c}", name=f"x_t{c}")
                   for c in range(HC)]
            ident_b = pers.tile([128, 128], BF16, tag="ident_b")
            nc.vector.tensor_copy(ident_b[:], ident[:])
            for lt in range(LT):
                xn = xin.tile([128, HIDDEN], F32, tag="x_nat")
                nc.gpsimd.dma_start(out=xn[:], in_=x_d[128 * lt:128 * (lt + 1), :])
                xnb = xin.tile([128, HIDDEN], BF16, tag="x_natb")
                nc.vector.tensor_copy(xnb[:], xn[:])
                for c in range(HC):
                    tp = ps2.tile([128, 128], BF16, tag="s", name=f"tr_ps{lt}_{c}",
                                  padded_shape=[128, 1024])
                    nc.tensor.transpose(tp[:], xnb[:, 128 * c:128 * (c + 1)],
                                        ident_b[:])
                    nc.vector.tensor_copy(
                        x_t[c][:, 128 * lt:128 * (lt + 1)], tp[:]
                    )

            # ---------- phase 1.5: residual slice x^T (fp32) ----------
            xr_t_a = pers.tile([128, L], F32, tag="xr_t_a")
            xr_t_b = pers.tile([64, L], F32, tag="xr_t_b")
            for lt in range(LT):
                xrn = xin.tile([128, OSL], F32, tag="xr_nat")
                nc.gpsimd.dma_start(out=xrn[:],
                                    in_=xr_d[128 * lt:128 * (lt + 1), :])
                tp = ps_tile([128, 128], f"xr_ps{lt}_0")
                nc.tensor.transpose(tp[:], xrn[:, 0:128], ident[:])
                nc.vector.tensor_copy(xr_t_a[:, 128 * lt:128 * (lt + 1)], tp[:])
                tp2 = ps_tile([64, 128], f"xr_ps{lt}_1")
                nc.tensor.transpose(tp2[:], xrn[:, 128:192], ident[:])
                nc.vector.tensor_copy(xr_t_b[:, 128 * lt:128 * (lt + 1)], tp2[:])

            # ---------- phase 2: QKV projections ----------
            # q^T / k^T: [192, L] as a [128, L] + [64, L] pair
            q_a = pers.tile([128, L], BF16, tag="q_a")
            k_a = pers.tile([128, L], BF16, tag="k_a")
            q_b_t = pers.tile([64, L], BF16, tag="q_b")
            k_b_t = pers.tile([64, L], BF16, tag="k_b")
            q_b = q_b_t[:]
            k_b = k_b_t[:]
            for wi, (dst, w_sb, bcol) in enumerate((
                ((q_a[:], q_b), wq, 0),
                ((k_a[:], k_b), wk, 2),
            )):
                for fc in range(2):  # feature chunk: 0 -> 128 rows, 1 -> 64 rows
                    m = 128 if fc == 0 else 64
                    for half in range(2):
                        ps = ps_tile([m, 1024], f"qk_ps{wi}_{fc}_{half}")
                        for qt in range(2):
                            sl = slice(512 * qt, 512 * (qt + 1))
                            xsl = slice(1024 * half + 512 * qt,
                                        1024 * half + 512 * (qt + 1))
                            for c in range(HC):
                                nc.tensor.matmul(
                                    ps[:, sl],
                                    w_sb[:, c, 128 * fc:128 * fc + m],
                                    x_t[c][:, xsl],
                                    start=(c == 0),
                                    stop=(c == HC - 1),
                                )
                        nc.vector.tensor_scalar_add(
                            dst[fc][:, 1024 * half:1024 * (half + 1)], ps[:],
                            pcol[0:m, bcol + fc:bcol + fc + 1]
                        )

            # V' tiles: [128, 3*65] per l-tile; per head h cols 65h..65h+63 =
            # (x@wv + b)*mask, col 65h+64 = mask
            v_sb = [work.tile([128, 3 * 65], BF16, tag=f"v{lt}", name=f"v{lt}",
                              bufs=1)
                    for lt in range(LT)]
            for lt in range(LT):
                vp = ps_tile([128, HF], f"v_ps{lt}")
                for c in range(HC):
                    nc.tensor.matmul(
                        vp[:],
                        x_t[c][:, 128 * lt:128 * (lt + 1)],
                        wv[:, c, :],
                        start=(c == 0),
                        stop=False,
                    )
                # + wv_b broadcast over rows: ones_col^T (K=1) x bias row
                nc.tensor.matmul(
                    vp[:],
                    ones_bf[:, 0:128],
                    prow[:, 0:HF],
                    start=False,
                    stop=True,
                )
                for h in range(HPC):
                    nc.vector.tensor_scalar_mul(
                        v_sb[lt][:, 65 * h:65 * h + 64],
                        vp[:, 64 * h:64 * (h + 1)],
                        mask_cols[:, lt:lt + 1],
                    )
                    nc.vector.tensor_copy(
                        v_sb[lt][:, 65 * h + 64:65 * h + 65],
                        mask_cols[:, lt:lt + 1],
                    )

            # ---------- phase 3+4+5: attention / projection / split RS ----
            attn_a = pers.tile([128, L], BF16, tag="attn_a")  # heads 0,1
            attn_b = pers.tile([64, L], BF16, tag="attn_b")   # head 2

            def attn_normalize(av, h, qh, o_ap):
                q0 = 1024 * qh
                av_sb = work.tile([64, 1024], F32, tag="av_sb", bufs=2,
                                  name=f"avs{h}_{qh}")
                nc.scalar.copy(av_sb[:], av[0:64, :])
                l_sb = work.tile([1, 1024], BF16, tag="l_sb", bufs=2,
                                 name=f"l{h}_{qh}")
                nc.scalar.copy(l_sb[:], av[64:65, :])
                rb = ps2.tile([64, 1024], F32, tag="s", name=f"rb{h}_{qh}",
                              padded_shape=[128, 1024])
                for i in range(2):
                    nc.tensor.matmul(
                        rb[:, 512 * i:512 * (i + 1)],
                        ones_bf[:, 0:64],
                        l_sb[:, 512 * i:512 * (i + 1)],
                    )
                rb_sb = work.tile([64, 1024], F32, tag="rb_sb", bufs=2,
                                  name=f"rbs{h}_{qh}")
                nc.vector.reciprocal(rb_sb[:], rb[:])
                nc.vector.tensor_mul(
                    rb_sb[:], rb_sb[:], mask_bc[:, q0:q0 + 1024]
                )
                nc.vector.tensor_mul(
                    o_ap[:, q0:q0 + 1024], av_sb[:], rb_sb[:]
                )

            partial_qh = [
                nc.dram_tensor("partial_q0", [HIDDEN, 1024], F32),
                nc.dram_tensor("partial_q1", [HIDDEN, 1024], F32),
            ]
            rs_qh = [
                nc.dram_tensor("rs_out_q0", [OSL * 1024], F32),
                nc.dram_tensor("rs_out_q1", [OSL * 1024], F32),
            ]
            for qh in range(2):
                q0 = 1024 * qh
                # heads 0,1: row-group-packed scores (K=64 pairs), shared
                # exp tiles [h0 512q | h1 512q]
                av0 = pav.tile([65, 1024], F32, tag="av", bufs=2,
                               name=f"av0_{qh}")
                av1 = pav.tile([65, 1024], F32, tag="av", bufs=2,
                               name=f"av1_{qh}")
                for kt in range(LT):
                    ksl = slice(128 * kt, 128 * (kt + 1))
                    ptiles = []
                    for qq in range(2):
                        qsl = slice(q0 + 512 * qq, q0 + 512 * (qq + 1))
                        sp = ps_tile([128, 1024], f"s01_{qh}_{kt}_{qq}")
                        nc.tensor.matmul(sp[:, 0:512], k_a[0:64, ksl],
                                         q_a[0:64, qsl])
                        nc.tensor.matmul(sp[:, 512:1024], k_a[64:128, ksl],
                                         q_a[64:128, qsl])
                        pexp_t = pexp.tile([128, 1024], BF16, tag="p",
                                           name=f"p01_{qh}_{kt}_{qq}")
                        nc.scalar.activation(pexp_t[:], sp[:], AF.Exp,
                                             scale=0.125)
                        ptiles.append(pexp_t)
                    for hh, av in ((0, av0), (1, av1)):
                        for qq in range(2):
                            nc.tensor.matmul(
                                av[:, 512 * qq:512 * (qq + 1)],
                                v_sb[kt][:, 65 * hh:65 * (hh + 1)],
                                ptiles[qq][:, 512 * hh:512 * (hh + 1)],
                                start=(kt == 0),
                                stop=(kt == LT - 1),
                            )
                attn_normalize(av0, 0, qh, attn_a[0:64, :])
                attn_normalize(av1, 1, qh, attn_a[64:128, :])
                # head 2 (solo)
                av2 = pav.tile([65, 1024], F32, tag="av", bufs=2,
                               name=f"av2_{qh}")
                for kt in range(LT):
                    ksl = slice(128 * kt, 128 * (kt + 1))
                    sp = ps_tile([128, 1024], f"s2_{qh}_{kt}")
                    for qq in range(2):
                        qsl = slice(q0 + 512 * qq, q0 + 512 * (qq + 1))
                        nc.tensor.matmul(sp[:, 512 * qq:512 * (qq + 1)],
                                         k_b[:, ksl], q_b[:, qsl])
                    pexp_t = pexp.tile([128, 1024], BF16, tag="p",
                                       name=f"p2_{qh}_{kt}")
                    nc.scalar.activation(pexp_t[:], sp[:], AF.Exp, scale=0.125)
                    for qq in range(2):
                        nc.tensor.matmul(
                            av2[:, 512 * qq:512 * (qq + 1)],
                            v_sb[kt][:, 130:195],
                            pexp_t[:, 512 * qq:512 * (qq + 1)],
                            start=(kt == 0),
                            stop=(kt == LT - 1),
                        )
                attn_normalize(av2, 2, qh, attn_b[:])

                # projection for this query half, then its ReduceScatter
                for oc in range(HC):
                    st = work.tile([128, 1024], F32, tag="stage", bufs=2,
                                   name=f"st{qh}_{oc}")
                    po = ps_tile([128, 1024], f"po{qh}_{oc}")
                    for qt in range(2):
                        sl = slice(512 * qt, 512 * (qt + 1))
                        asl = slice(q0 + 512 * qt, q0 + 512 * (qt + 1))
                        nc.tensor.matmul(
                            po[:, sl],
                            wo_a[:, 128 * oc:128 * (oc + 1)],
                            attn_a[:, asl],
                            start=True,
                            stop=False,
                        )
                        nc.tensor.matmul(
                            po[:, sl],
                            wo_b_sb[:, 128 * oc:128 * (oc + 1)],
                            attn_b[:, asl],
                            start=False,
                            stop=False,
                        )
                        # + wo_b/4 broadcast over columns
                        nc.tensor.matmul(
                            po[:, sl],
                            prow[:, HF + 128 * oc:HF + 128 * (oc + 1)],
                            ones_bf[:, 0:512],
                            start=False,
                            stop=True,
                        )
                    nc.vector.tensor_copy(st[:], po[:])
                    nc.gpsimd.dma_start(
                        out=partial_qh[qh][128 * oc:128 * (oc + 1), :],
                        in_=st[:],
                    )
                nc.gpsimd.collective_compute(
                    "ReduceScatter",
                    ALU.add,
                    replica_groups=[[0, 1, 2, 3], [4, 5, 6, 7]],
                    ins=[partial_qh[qh][:].opt()],
                    outs=[rs_qh[qh][:].opt()],
                )

            # ---------- layernorm over L ----------
            for pc, m in ((0, 128), (1, 64)):
                xr_ap = xr_t_a[:] if pc == 0 else xr_t_b[:]
                y = work.tile([m, L], F32, tag="y", bufs=1, name=f"y{pc}")
                bnst = work.tile([m, 4 * 6], F32, tag=f"bnst{pc}", bufs=1,
                                 name=f"bnst{pc}")
                for qh in range(2):
                    rs_ap = rs_qh[qh][:].rearrange("(r l) -> r l", l=1024)
                    yb = work.tile([m, 1024], F32, tag="yb", bufs=2,
                                   name=f"yb{pc}_{qh}")
                    nc.sync.dma_start(out=yb[:],
                                      in_=rs_ap[128 * pc:128 * pc + m, :])
                    nc.vector.tensor_add(
                        y[:, 1024 * qh:1024 * (qh + 1)],
                        xr_ap[:, 1024 * qh:1024 * (qh + 1)], yb[:]
                    )
                    for cch in range(2):
                        nc.vector.bn_stats(
                            bnst[:, 6 * (2 * qh + cch):6 * (2 * qh + cch + 1)],
                            y[:, 1024 * qh + 512 * cch:
                              1024 * qh + 512 * (cch + 1)],
                        )
                stats = work.tile([m, 2], F32, tag=f"stats{pc}", bufs=1,
                                  name=f"stats{pc}")
                nc.vector.bn_aggr(stats[:], bnst[:])
                std = work.tile([m, 1], F32, tag=f"std{pc}", bufs=1,
                                name=f"std{pc}")
                nc.scalar.activation(
                    std[:], stats[:, 1:2], AF.Sqrt, scale=float(L) / float(L - 1)
                )
                rstd = work.tile([m, 1], F32, tag=f"rstd{pc}", bufs=1,
                                 name=f"rstd{pc}")
                nc.vector.reciprocal(rstd[:], std[:])
                ga = pcol[0:m, 12 + pc:13 + pc]
                be = pcol[0:m, 14 + pc:15 + pc]
                amul = work.tile([m, 1], F32, tag=f"amul{pc}", bufs=1,
                                 name=f"amul{pc}")
                nc.vector.tensor_mul(amul[:], rstd[:], ga)
                tmpb = work.tile([m, 1], F32, tag=f"tmpb{pc}", bufs=1,
                                 name=f"tmpb{pc}")
                nc.vector.tensor_mul(tmpb[:], stats[:, 0:1], amul[:])
                badd = work.tile([m, 1], F32, tag=f"badd{pc}", bufs=1,
                                 name=f"badd{pc}")
                nc.vector.tensor_sub(badd[:], be, tmpb[:])
                yo = work.tile([m, L], F32, tag="yo", bufs=1,
                               name=f"yo{pc}")
                nc.vector.tensor_scalar(
                    yo[:], y[:], amul[:], badd[:], op0=ALU.mult, op1=ALU.add
                )
                nc.sync.dma_start(out=out_d[128 * pc:128 * pc + m, :], in_=yo[:])

    nc.compile()
    return nc


_NC = None


def _get_nc():
    global _NC
    if _NC is None:
        _NC = build_nc()
    return _NC


def make_in_maps(inputs, attention_mask, wq_w, wq_b, wk_w, wk_b, wv_w, wv_b,
                 wo_w, wo_b, gamma, beta):
    x = np.asarray(inputs, np.float32)
    am = np.asarray(attention_mask, np.int32)
    in_maps = []
    for c in range(NCORES):
        b, g = c // 4, c % 4
        hsl = slice(HF * g, HF * (g + 1))
        pcol = np.zeros((128, 16), np.float32)
        for j, vec in ((0, np.asarray(wq_b)[hsl]), (2, np.asarray(wk_b)[hsl]),
                       (4, np.asarray(wv_b)[hsl])):
            pcol[:, j] = vec[:128]
            pcol[:64, j + 1] = vec[128:]
        wob4 = np.asarray(wo_b, np.float32) / 4.0
        pcol[:, 6:12] = wob4.reshape(6, 128).T
        for j, vec in ((12, np.asarray(gamma)[hsl]), (14, np.asarray(beta)[hsl])):
            pcol[:, j] = vec[:128]
            pcol[:64, j + 1] = vec[128:]
        prow = np.zeros((1, 960), BFNP)
        prow[0, :HF] = np.asarray(wv_b)[hsl]
        prow[0, HF:] = wob4
        in_maps.append({
            "x": np.ascontiguousarray(x[b]),
            "xr": np.ascontiguousarray(x[b][:, hsl]),
            "wq": np.ascontiguousarray(np.asarray(wq_w, np.float32)[:, hsl].astype(BFNP)),
            "wk": np.ascontiguousarray(np.asarray(wk_w, np.float32)[:, hsl].astype(BFNP)),
            "wv": np.ascontiguousarray(np.asarray(wv_w, np.float32)[:, hsl].astype(BFNP)),
            "wo_r": np.ascontiguousarray(np.asarray(wo_w, np.float32)[hsl, :].astype(BFNP)),
            "mask_i": np.ascontiguousarray(am[b][None, :]),
            "params_col": pcol,
            "params_row": prow,
        })
    return in_maps


def run(trace=False, **inputs):
    nc = _get_nc()
    in_maps = make_in_maps(**inputs)
    res = run_bass_kernel_spmd(nc, in_maps, core_ids=list(range(NCORES)),
                               trace=trace)
    out = np.zeros((B, L, HIDDEN), np.float32)
    for c in range(NCORES):
        b, g = c // 4, c % 4
        out[b, :, HF * g:HF * (g + 1)] = res.results[c]["out_t"].T
    return out, res


def kernel(**inputs):
    out, _ = run(trace=False, **inputs)
    return out


# revision 29
# speedup vs baseline: 2.2025x; 1.0333x over previous
"""Trainium2 Bass kernel for nn_MultiHeadAttention (B=2, L=2048, H=768, 12 heads).

Sharding (8 cores): core c -> batch b=c//4, heads 3*(c%4)..3*(c%4)+2.
Each core: QKV proj for its 3 heads, flash-style attention (scores^T layout,
key-mask folded into V', query-mask folded into 1/l), partial output
projection with wo rows (row-parallel) + x/4 residual, ReduceScatter(add)
over the 4 cores of its batch, then layernorm over the sequence dim on its
192-row hidden slice. Host assembles [2,2048,768] from 8 [192,2048] slices.

PSUM static budget (8 banks): tag s = 2 bufs x [128,1024] (4 banks, shared by
scores / transposes / projections), tag av = [65,1024] (2), tag rb = [64,1024]
(2).
"""

import sys

import ml_dtypes
import numpy as np

BFNP = ml_dtypes.bfloat16

sys.path.insert(0, "/opt/trn_rl_repo")

import concourse.bass as bass  # noqa: E402
import concourse.bacc as bacc  # noqa: E402
import concourse.mybir as mybir  # noqa: E402
from concourse import tile  # noqa: E402
from concourse.bass_utils import run_bass_kernel_spmd  # noqa: E402

F32 = mybir.dt.float32
BF16 = mybir.dt.bfloat16
I32 = mybir.dt.int32
AF = mybir.ActivationFunctionType
ALU = mybir.AluOpType

HIDDEN = 768
HEADS = 12
HD = 64
L = 2048
B = 2
NCORES = 8
HPC = 3          # heads per core
HF = HPC * HD    # 192 features per core
LT = L // 128    # 16 l-tiles
HC = HIDDEN // 128  # 6 hidden chunks
OSL = HIDDEN // 4   # 192 output-slice rows per core


def build_nc():
    nc = bacc.Bacc("TRN2", target_bir_lowering=False, debug=False,
                   num_devices=NCORES)

    x_d = nc.dram_tensor("x", [L, HIDDEN], F32, kind="ExternalInput")
    wq_d = nc.dram_tensor("wq", [HIDDEN, HF], BF16, kind="ExternalInput")
    wk_d = nc.dram_tensor("wk", [HIDDEN, HF], BF16, kind="ExternalInput")
    wv_d = nc.dram_tensor("wv", [HIDDEN, HF], BF16, kind="ExternalInput")
    wo_d = nc.dram_tensor("wo_r", [HF, HIDDEN], BF16, kind="ExternalInput")
    mask_d = nc.dram_tensor("mask_i", [1, L], I32, kind="ExternalInput")
    # params_col[128, 16]: cols 0,1=wq_b(192) 2,3=wk_b 4,5=wv_b 6..11=wo_b/4
    # (768), 12,13=gamma slice, 14,15=beta slice
    pcol_d = nc.dram_tensor("params_col", [128, 16], F32, kind="ExternalInput")
    # params_row[1, 960]: 0:192 wv_b, 192:960 wo_b/4
    prow_d = nc.dram_tensor("params_row", [1, 960], BF16, kind="ExternalInput")
    xr_d = nc.dram_tensor("xr", [L, OSL], F32, kind="ExternalInput")
    out_d = nc.dram_tensor("out_t", [OSL, L], F32, kind="ExternalOutput")

    partial_d = nc.dram_tensor("partial_acc", [HIDDEN, L], F32)
    rs_d = nc.dram_tensor("rs_out", [OSL * L], F32)

    with tile.TileContext(nc) as tc:
        with (
            tc.tile_pool(name="persist", bufs=1) as pers,
            tc.tile_pool(name="xin", bufs=3) as xin,
            tc.tile_pool(name="work", bufs=2) as work,
            tc.tile_pool(name="ps2", bufs=2, space=bass.MemorySpace.PSUM) as ps2,
            tc.tile_pool(name="pav", bufs=2, space=bass.MemorySpace.PSUM) as pav,
            tc.tile_pool(name="pexp", bufs=3) as pexp,
        ):
            def ps_tile(shape, name):
                return ps2.tile(shape, F32, tag="s", name=name,
                                padded_shape=[128, 1024])

            # ---------- phase 0: constants ----------
            ident_i = pers.tile([128, 128], I32, tag="ident_i")
            nc.gpsimd.iota(ident_i[:], pattern=[[-1, 128]], base=0,
                           channel_multiplier=1)
            ident = pers.tile([128, 128], F32, tag="ident")
            nc.vector.tensor_scalar(
                ident[:], ident_i[:], 0, None, op0=ALU.is_equal
            )
            ones_row = pers.tile([1, 512], F32, tag="ones_row")
            nc.vector.memset(ones_row[:], 1.0)
            ones_bf = pers.tile([1, 512], BF16, tag="ones_bf")
            nc.vector.memset(ones_bf[:], 1.0)

            pcol = pers.tile([128, 16], F32, tag="pcol")
            nc.sync.dma_start(out=pcol[:], in_=pcol_d[:])
            prow = pers.tile([1, 960], BF16, tag="prow")
            nc.sync.dma_start(out=prow[:], in_=prow_d[:])

            mask_i = xin.tile([1, L], I32, tag="mask_i", bufs=1)
            nc.sync.dma_start(out=mask_i[:], in_=mask_d[:])
            mask_row = pers.tile([1, L], F32, tag="mask_row")
            nc.vector.tensor_copy(mask_row[:], mask_i[:])

            # mask columns [128, 16]: col t = mask[128t:128t+128]
            mask_cols = pers.tile([128, LT], F32, tag="mask_cols")
            for t in range(LT):
                mp = ps_tile([128, 1], f"mask_ps{t}")
                nc.tensor.matmul(
                    mp[:], mask_row[:, 128 * t:128 * (t + 1)], ones_row[:, 0:1]
                )
                nc.vector.tensor_copy(mask_cols[:, t:t + 1], mp[:])

            # query-mask broadcast over 64 partitions, built once
            mask_bc = pers.tile([64, L], BF16, tag="mask_bc")
            for i in range(2):
                mb = ps_tile([64, 1024], f"mb{i}")
                for j in range(2):
                    nc.tensor.matmul(
                        mb[:, 512 * j:512 * (j + 1)],
                        ones_row[:, 0:64],
                        mask_row[:, 1024 * i + 512 * j:1024 * i + 512 * (j + 1)],
                    )
                nc.vector.tensor_copy(mask_bc[:, 1024 * i:1024 * (i + 1)], mb[:])

            # weights loaded early; tiny PE "touch" matmuls absorb each DMA
            # lane wait so later matmuls stay under the 2-wait limit
            wq = pers.tile([128, HC, HF], BF16, tag="wq")
            wk = pers.tile([128, HC, HF], BF16, tag="wk")
            wv = pers.tile([128, HC, HF], BF16, tag="wv")
            for w_sb, w_d in ((wq, wq_d), (wk, wk_d), (wv, wv_d)):
                nc.sync.dma_start(
                    out=w_sb[:], in_=w_d[:].rearrange("(c p) m -> p c m", p=128)
                )
            wo_a = pers.tile([128, HIDDEN], BF16, tag="wo_a")
            wo_b_sb = pers.tile([64, HIDDEN], BF16, tag="wo_b")
            nc.sync.dma_start(out=wo_a[:], in_=wo_d[0:128, :])
            nc.sync.dma_start(out=wo_b_sb[:], in_=wo_d[128:192, :])
            touch_srcs = (wq[:, 0, 0:1], wk[:, 0, 0:1], wv[:, 0, 0:1],
                          wo_a[:, 0:1], wo_b_sb[:, 0:1], prow[:, 0:1])
            tch = pav.tile([1, 1], F32, tag="av", name="touch",
                           padded_shape=[65, 1024])
            for ti, tsr in enumerate(touch_srcs):
                nc.tensor.matmul(tch[:], tsr, tsr, start=(ti == 0),
                                 stop=(ti == len(touch_srcs) - 1),
                                 skip_group_check=True)
            tch_scr = work.tile([1, 1], F32, tag="tch_scr", bufs=1)
            nc.scalar.copy(tch_scr[:], tch[:])

            # ---------- phase 1: load x, build x^T ----------
            x_t = [pers.tile([128, L], BF16, tag=f"x_t{c}", name=f"x_t{c}")
                   for c in range(HC)]
            ident_b = pers.tile([128, 128], BF16, tag="ident_b")
            nc.vector.tensor_copy(ident_b[:], ident[:])
            for lt in range(LT):
                xn = xin.tile([128, HIDDEN], F32, tag="x_nat")
                nc.gpsimd.dma_start(out=xn[:], in_=x_d[128 * lt:128 * (lt + 1), :])
                xnb = xin.tile([128, HIDDEN], BF16, tag="x_natb")
                nc.vector.tensor_copy(xnb[:], xn[:])
                for c in range(HC):
                    tp = ps2.tile([128, 128], BF16, tag="s", name=f"tr_ps{lt}_{c}",
                                  padded_shape=[128, 1024])
                    nc.tensor.transpose(tp[:], xnb[:, 128 * c:128 * (c + 1)],
                                        ident_b[:])
                    nc.vector.tensor_copy(
                        x_t[c][:, 128 * lt:128 * (lt + 1)], tp[:]
                    )

            # ---------- phase 1.5: residual slice x^T (fp32) ----------
            xr_t_a = pers.tile([128, L], F32, tag="xr_t_a")
            xr_t_b = pers.tile([64, L], F32, tag="xr_t_b")
            for lt in range(LT):
                xrn = xin.tile([128, OSL], F32, tag="xr_nat")
                nc.gpsimd.dma_start(out=xrn[:],
                                    in_=xr_d[128 * lt:128 * (lt + 1), :])
                tp = ps_tile([128, 128], f"xr_ps{lt}_0")
                nc.tensor.transpose(tp[:], xrn[:, 0:128], ident[:])
                nc.vector.tensor_copy(xr_t_a[:, 128 * lt:128 * (lt + 1)], tp[:])
                tp2 = ps_tile([64, 128], f"xr_ps{lt}_1")
                nc.tensor.transpose(tp2[:], xrn[:, 128:192], ident[:])
                nc.vector.tensor_copy(xr_t_b[:, 128 * lt:128 * (lt + 1)], tp2[:])

            # ---------- phase 2: QKV projections ----------
            # q^T / k^T: [192, L] as a [128, L] + [64, L] pair
            q_a = pers.tile([128, L], BF16, tag="q_a")
            k_a = pers.tile([128, L], BF16, tag="k_a")
            q_b_t = pers.tile([64, L], BF16, tag="q_b")
            k_b_t = pers.tile([64, L], BF16, tag="k_b")
            q_b = q_b_t[:]
            k_b = k_b_t[:]
            for wi, (dst, w_sb, bcol) in enumerate((
                ((q_a[:], q_b), wq, 0),
                ((k_a[:], k_b), wk, 2),
            )):
                for fc in range(2):  # feature chunk: 0 -> 128 rows, 1 -> 64 rows
                    m = 128 if fc == 0 else 64
                    for half in range(2):
                        ps = ps_tile([m, 1024], f"qk_ps{wi}_{fc}_{half}")
                        for qt in range(2):
                            sl = slice(512 * qt, 512 * (qt + 1))
                            xsl = slice(1024 * half + 512 * qt,
                                        1024 * half + 512 * (qt + 1))
                            for c in range(HC):
                                nc.tensor.matmul(
                                    ps[:, sl],
                                    w_sb[:, c, 128 * fc:128 * fc + m],
                                    x_t[c][:, xsl],
                                    start=(c == 0),
                                    stop=(c == HC - 1),
                                )
                        nc.vector.tensor_scalar_add(
                            dst[fc][:, 1024 * half:1024 * (half + 1)], ps[:],
                            pcol[0:m, bcol + fc:bcol + fc + 1]
                        )

            # V' tiles: [128, 3*65] per l-tile; per head h cols 65h..65h+63 =
            # (x@wv + b)*mask, col 65h+64 = mask
            v_sb = [work.tile([128, 3 * 65], BF16, tag=f"v{lt}", name=f"v{lt}",
                              bufs=1)
                    for lt in range(LT)]
            for lt in range(LT):
                vp = ps_tile([128, HF], f"v_ps{lt}")
                for c in range(HC):
                    nc.tensor.matmul(
                        vp[:],
                        x_t[c][:, 128 * lt:128 * (lt + 1)],
                        wv[:, c, :],
                        start=(c == 0),
                        stop=False,
                    )
                # + wv_b broadcast over rows: ones_col^T (K=1) x bias row
                nc.tensor.matmul(
                    vp[:],
                    ones_bf[:, 0:128],
                    prow[:, 0:HF],
                    start=False,
                    stop=True,
                )
                for h in range(HPC):
                    nc.vector.tensor_scalar_mul(
                        v_sb[lt][:, 65 * h:65 * h + 64],
                        vp[:, 64 * h:64 * (h + 1)],
                        mask_cols[:, lt:lt + 1],
                    )
                    nc.vector.tensor_copy(
                        v_sb[lt][:, 65 * h + 64:65 * h + 65],
                        mask_cols[:, lt:lt + 1],
                    )

            # ---------- phase 3+4+5: attention / projection / split RS ----
            attn_a = pers.tile([128, L], BF16, tag="attn_a")  # heads 0,1
            attn_b = pers.tile([64, L], BF16, tag="attn_b")   # head 2

            def attn_normalize(av, h, qh, o_ap):
                q0 = 1024 * qh
                av_sb = work.tile([64, 1024], F32, tag="av_sb", bufs=2,
                                  name=f"avs{h}_{qh}")
                nc.scalar.copy(av_sb[:], av[0:64, :])
                l_sb = work.tile([1, 1024], F32, tag="l_sb", bufs=2,
                                 name=f"l{h}_{qh}")
                nc.scalar.copy(l_sb[:], av[64:65, :])
                r_row = work.tile([1, 1024], F32, tag="r_row", bufs=2,
                                  name=f"rr{h}_{qh}")
                nc.vector.reciprocal(r_row[:], l_sb[:])
                rb_sb = work.tile([64, 1024], F32, tag="rb_sb", bufs=2,
                                  name=f"rbs{h}_{qh}")
                nc.gpsimd.partition_broadcast(rb_sb[:], r_row[:])
                nc.vector.tensor_mul(
                    rb_sb[:], rb_sb[:], mask_bc[:, q0:q0 + 1024]
                )
                nc.vector.tensor_mul(
                    o_ap[:, q0:q0 + 1024], av_sb[:], rb_sb[:]
                )

            ln_state = {}

            def ln_chunk(qh):
                for pc, m in ((0, 128), (1, 64)):
                    xr_ap = xr_t_a[:] if pc == 0 else xr_t_b[:]
                    if qh == 0 and pc == 0:
                        ln_state['y0'] = work.tile([128, L], F32, tag="y0",
                                                   bufs=1, name="y0")
                        ln_state['y1'] = work.tile([64, L], F32, tag="y1",
                                                   bufs=1, name="y1")
                        ln_state['bn0'] = work.tile([128, 24], F32, tag="bn0",
                                                    bufs=1, name="bn0")
                        ln_state['bn1'] = work.tile([64, 24], F32, tag="bn1",
                                                    bufs=1, name="bn1")
                    y = ln_state[f'y{pc}']
                    bnst = ln_state[f'bn{pc}']
                    rs_ap = rs_qh[qh][:].rearrange("(r l) -> r l", l=1024)
                    yb = work.tile([m, 1024], F32, tag="yb", bufs=2,
                                   name=f"yb{pc}_{qh}")
                    nc.sync.dma_start(out=yb[:],
                                      in_=rs_ap[128 * pc:128 * pc + m, :])
                    nc.vector.tensor_add(
                        y[:, 1024 * qh:1024 * (qh + 1)],
                        xr_ap[:, 1024 * qh:1024 * (qh + 1)], yb[:]
                    )
                    for cch in range(2):
                        nc.vector.bn_stats(
                            bnst[:, 6 * (2 * qh + cch):6 * (2 * qh + cch + 1)],
                            y[:, 1024 * qh + 512 * cch:
                              1024 * qh + 512 * (cch + 1)],
                        )

            partial_qh = [
                nc.dram_tensor("partial_q0", [HIDDEN, 1024], F32),
                nc.dram_tensor("partial_q1", [HIDDEN, 1024], F32),
            ]
            rs_qh = [
                nc.dram_tensor("rs_out_q0", [OSL * 1024], F32),
                nc.dram_tensor("rs_out_q1", [OSL * 1024], F32),
            ]
            for qh in range(2):
                q0 = 1024 * qh
                # heads 0,1: row-group-packed scores (K=64 pairs), shared
                # exp tiles [h0 512q | h1 512q]
                av0 = pav.tile([65, 1024], F32, tag="av", bufs=2,
                               name=f"av0_{qh}")
                av1 = pav.tile([65, 1024], F32, tag="av", bufs=2,
                               name=f"av1_{qh}")
                for kt in range(LT):
                    ksl = slice(128 * kt, 128 * (kt + 1))
                    ptiles = []
                    for qq in range(2):
                        qsl = slice(q0 + 512 * qq, q0 + 512 * (qq + 1))
                        sp = ps_tile([128, 1024], f"s01_{qh}_{kt}_{qq}")
                        nc.tensor.matmul(sp[:, 0:512], k_a[0:64, ksl],
                                         q_a[0:64, qsl])
                        nc.tensor.matmul(sp[:, 512:1024], k_a[64:128, ksl],
                                         q_a[64:128, qsl])
                        pexp_t = pexp.tile([128, 1024], BF16, tag="p",
                                           name=f"p01_{qh}_{kt}_{qq}")
                        nc.scalar.activation(pexp_t[:], sp[:], AF.Exp,
                                             scale=0.125)
                        ptiles.append(pexp_t)
                    for hh, av in ((0, av0), (1, av1)):
                        for qq in range(2):
                            nc.tensor.matmul(
                                av[:, 512 * qq:512 * (qq + 1)],
                                v_sb[kt][:, 65 * hh:65 * (hh + 1)],
                                ptiles[qq][:, 512 * hh:512 * (hh + 1)],
                                start=(kt == 0),
                                stop=(kt == LT - 1),
                            )
                attn_normalize(av0, 0, qh, attn_a[0:64, :])
                attn_normalize(av1, 1, qh, attn_a[64:128, :])
                # head 2 (solo)
                av2 = pav.tile([65, 1024], F32, tag="av", bufs=2,
                               name=f"av2_{qh}")
                for kt in range(LT):
                    ksl = slice(128 * kt, 128 * (kt + 1))
                    sp = ps_tile([128, 1024], f"s2_{qh}_{kt}")
                    for qq in range(2):
                        qsl = slice(q0 + 512 * qq, q0 + 512 * (qq + 1))
                        nc.tensor.matmul(sp[:, 512 * qq:512 * (qq + 1)],
                                         k_b[:, ksl], q_b[:, qsl])
                    pexp_t = pexp.tile([128, 1024], BF16, tag="p",
                                       name=f"p2_{qh}_{kt}")
                    nc.scalar.activation(pexp_t[:], sp[:], AF.Exp, scale=0.125)
                    for qq in range(2):
                        nc.tensor.matmul(
                            av2[:, 512 * qq:512 * (qq + 1)],
                            v_sb[kt][:, 130:195],
                            pexp_t[:, 512 * qq:512 * (qq + 1)],
                            start=(kt == 0),
                            stop=(kt == LT - 1),
                        )
                attn_normalize(av2, 2, qh, attn_b[:])

                if qh == 1:
                    # half-0 layernorm chunk: RS0 finished during qh1's
                    # attention; emit here so the DVE queue stays clear
                    ln_chunk(0)

                # projection for this query half, then its ReduceScatter
                for oc in range(HC):
                    st = work.tile([128, 1024], F32, tag="stage", bufs=2,
                                   name=f"st{qh}_{oc}")
                    po = ps_tile([128, 1024], f"po{qh}_{oc}")
                    for qt in range(2):
                        sl = slice(512 * qt, 512 * (qt + 1))
                        asl = slice(q0 + 512 * qt, q0 + 512 * (qt + 1))
                        nc.tensor.matmul(
                            po[:, sl],
                            wo_a[:, 128 * oc:128 * (oc + 1)],
                            attn_a[:, asl],
                            start=True,
                            stop=False,
                        )
                        nc.tensor.matmul(
                            po[:, sl],
                            wo_b_sb[:, 128 * oc:128 * (oc + 1)],
                            attn_b[:, asl],
                            start=False,
                            stop=False,
                        )
                        # + wo_b/4 broadcast over columns
                        nc.tensor.matmul(
                            po[:, sl],
                            prow[:, HF + 128 * oc:HF + 128 * (oc + 1)],
                            ones_bf[:, 0:512],
                            start=False,
                            stop=True,
                        )
                    nc.vector.tensor_copy(st[:], po[:])
                    nc.gpsimd.dma_start(
                        out=partial_qh[qh][128 * oc:128 * (oc + 1), :],
                        in_=st[:],
                    )
                nc.gpsimd.collective_compute(
                    "ReduceScatter",
                    ALU.add,
                    replica_groups=[[0, 1, 2, 3], [4, 5, 6, 7]],
                    ins=[partial_qh[qh][:].opt()],
                    outs=[rs_qh[qh][:].opt()],
                )

            # ---------- layernorm over L (second half + finish) ----------
            ln_chunk(1)
            for pc, m in ((0, 128), (1, 64)):
                y = ln_state[f'y{pc}']
                bnst = ln_state[f'bn{pc}']
                stats = work.tile([m, 2], F32, tag=f"stats{pc}", bufs=1,
                                  name=f"stats{pc}")
                nc.vector.bn_aggr(stats[:], bnst[:])
                std = work.tile([m, 1], F32, tag=f"std{pc}", bufs=1,
                                name=f"std{pc}")
                nc.scalar.activation(
                    std[:], stats[:, 1:2], AF.Sqrt, scale=float(L) / float(L - 1)
                )
                rstd = work.tile([m, 1], F32, tag=f"rstd{pc}", bufs=1,
                                 name=f"rstd{pc}")
                nc.vector.reciprocal(rstd[:], std[:])
                ga = pcol[0:m, 12 + pc:13 + pc]
                be = pcol[0:m, 14 + pc:15 + pc]
                amul = work.tile([m, 1], F32, tag=f"amul{pc}", bufs=1,
                                 name=f"amul{pc}")
                nc.vector.tensor_mul(amul[:], rstd[:], ga)
                tmpb = work.tile([m, 1], F32, tag=f"tmpb{pc}", bufs=1,
                                 name=f"tmpb{pc}")
                nc.vector.tensor_mul(tmpb[:], stats[:, 0:1], amul[:])
                badd = work.tile([m, 1], F32, tag=f"badd{pc}", bufs=1,
                                 name=f"badd{pc}")
                nc.vector.tensor_sub(badd[:], be, tmpb[:])
                yo = work.tile([m, L], F32, tag="yo", bufs=1,
                               name=f"yo{pc}")
                nc.vector.tensor_scalar(
                    yo[:], y[:], amul[:], badd[:], op0=ALU.mult, op1=ALU.add
                )
                nc.sync.dma_start(out=out_d[128 * pc:128 * pc + m, :], in_=yo[:])

    nc.compile()
    return nc


_NC = None


def _get_nc():
    global _NC
    if _NC is None:
        _NC = build_nc()
    return _NC


def make_in_maps(inputs, attention_mask, wq_w, wq_b, wk_w, wk_b, wv_w, wv_b,
                 wo_w, wo_b, gamma, beta):
    x = np.asarray(inputs, np.float32)
    am = np.asarray(attention_mask, np.int32)
    in_maps = []
    for c in range(NCORES):
        b, g = c // 4, c % 4
        hsl = slice(HF * g, HF * (g + 1))
        pcol = np.zeros((128, 16), np.float32)
        for j, vec in ((0, np.asarray(wq_b)[hsl]), (2, np.asarray(wk_b)[hsl]),
                       (4, np.asarray(wv_b)[hsl])):
            pcol[:, j] = vec[:128]
            pcol[:64, j + 1] = vec[128:]
        wob4 = np.asarray(wo_b, np.float32) / 4.0
        pcol[:, 6:12] = wob4.reshape(6, 128).T
        for j, vec in ((12, np.asarray(gamma)[hsl]), (14, np.asarray(beta)[hsl])):
            pcol[:, j] = vec[:128]
            pcol[:64, j + 1] = vec[128:]
        prow = np.zeros((1, 960), BFNP)
        prow[0, :HF] = np.asarray(wv_b)[hsl]
        prow[0, HF:] = wob4
        in_maps.append({
            "x": np.ascontiguousarray(x[b]),
            "xr": np.ascontiguousarray(x[b][:, hsl]),
            "wq": np.ascontiguousarray(np.asarray(wq_w, np.float32)[:, hsl].astype(BFNP)),
            "wk": np.ascontiguousarray(np.asarray(wk_w, np.float32)[:, hsl].astype(BFNP)),
            "wv": np.ascontiguousarray(np.asarray(wv_w, np.float32)[:, hsl].astype(BFNP)),
            "wo_r": np.ascontiguousarray(np.asarray(wo_w, np.float32)[hsl, :].astype(BFNP)),
            "mask_i": np.ascontiguousarray(am[b][None, :]),
            "params_col": pcol,
            "params_row": prow,
        })
    return in_maps


def run(trace=False, **inputs):
    nc = _get_nc()
    in_maps = make_in_maps(**inputs)
    res = run_bass_kernel_spmd(nc, in_maps, core_ids=list(range(NCORES)),
                               trace=trace)
    out = np.zeros((B, L, HIDDEN), np.float32)
    for c in range(NCORES):
        b, g = c // 4, c % 4
        out[b, :, HF * g:HF * (g + 1)] = res.results[c]["out_t"].T
    return out, res


def kernel(**inputs):
    out, _ = run(trace=False, **inputs)
    return out
